# revision 14
# baseline (speedup 1.0000x reference)
"""Trainium2 Bass kernel for nn_EncoderBlock (dense transformer encoder block).

Data parallel: batch B=8 across 8 NeuronCores, one element per core.

v2 design vs v1 baseline (685us):
  - scores matmuls in fp8 DoubleRow (zero-padded K=64 subtile): 2x.
  - ctx computed "flipped" (out = [queries, feats]) in fp8 DoubleRow with a
    fused ones-column producing the softmax normalizer Z in the same psum
    tile; per-partition normalize via reciprocal_approx_fast + tensor_scalar.
    Kills the ones-matmul for Z and the replicated [128,512] reciprocal.
  - wo in fp8 DoubleRow; wo weights pre-scaled x64 host-side and ctx scaled
    x16 on-chip (fp8 subnormal avoidance), un-scaled by 1/1024 in the
    residual add.
  - qproj per head-pair pipelined under the attention-half-0 exp stream;
    ffn1(half 0) interleaved under the attention-half-1 exp stream.
  - LN sum/sum-of-squares both via ScalarE ACT accumulate (Copy / Square).
FFN and qproj matmuls stay bf16 (fp8 there would blow the 2e-2 error gate).
"""

import sys

sys.path.insert(0, "/opt/trn_rl_repo")

import numpy as np
import ml_dtypes
from contextlib import ExitStack

import concourse.bass as bass
import concourse.tile as tile
from concourse import bacc, mybir
from concourse import bass_utils
from concourse.bass import ts, ds
from concourse.masks import make_identity

BF = mybir.dt.bfloat16
F32 = mybir.dt.float32
FP8 = mybir.dt.float8e4
AF = mybir.ActivationFunctionType
OP = mybir.AluOpType
DR = mybir.MatmulPerfMode.DoubleRow

P = 128
S = 1024          # sequence length per core
D = 1024          # d_model
H = 16            # heads
DK = 64           # head dim
DFF = 4096
NB = 8            # batch = number of cores
SC = S // P       # 8 sequence chunks
DC = D // P       # 8 feature chunks
FC = DFF // P     # 32 ff chunks
EPS = 1e-6
EXP_SHIFT = -2.0   # constant shift inside exp; cancels in softmax ratio
CTX_SCALE = 16.0   # on-chip scale of v (=q) into qh8e; keeps ctx out of
WO_SCALE = 64.0    # fp8 subnormal range.  woT8 = fp8(64*wo) host-side.
OUT_SCALE = 1.0 / (CTX_SCALE * WO_SCALE)
HW = 66            # per-head stride in qh8e: 64 feats + Z-ones col + pad

last_exec_time_ns = None


def _emit_ln_chunk(nc, small, x_ap, out_ap, scratch_ap, alpha, beta,
                   apply_on_scalar=False):
    """Bessel-corrected LN of one [P, D] chunk, stats per token on partitions.
    n = (x - mu)/(std + eps)*alpha + beta.  Sum and sum-of-squares come from
    ScalarE ACT accumulate (Copy / Square) writing scratch_ap; scratch is
    overwritten by the final apply (out_ap may alias scratch_ap)."""
    s1 = small.tile([P, 1], F32, tag="ln_s1", bufs=3, name="ln_s1")
    sq = small.tile([P, 1], F32, tag="ln_sq", bufs=3, name="ln_sq")
    mu = small.tile([P, 1], F32, tag="ln_mu", bufs=3, name="ln_mu")
    var = small.tile([P, 1], F32, tag="ln_var", bufs=3, name="ln_var")
    tmp = small.tile([P, 1], F32, tag="ln_tmp", bufs=3, name="ln_tmp")
    s0 = small.tile([P, 1], F32, tag="ln_s0", bufs=3, name="ln_s0")
    tc_ = small.tile([P, 1], F32, tag="ln_tc", bufs=3, name="ln_tc")
    uc_ = small.tile([P, 1], F32, tag="ln_uc", bufs=3, name="ln_uc")

    nc.scalar.activation(scratch_ap, x_ap, AF.Copy, accum_out=s1[:])
    nc.scalar.activation(scratch_ap, x_ap, AF.Square, accum_out=sq[:])
    nc.vector.tensor_scalar_mul(mu[:], s1[:], 1.0 / D)
    nc.vector.tensor_mul(tmp[:], mu[:], mu[:])
    nc.vector.tensor_scalar_mul(var[:], sq[:], 1.0 / (D - 1))
    nc.vector.tensor_scalar_mul(tmp[:], tmp[:], float(D) / (D - 1))
    nc.vector.tensor_sub(var[:], var[:], tmp[:])
    # std = sqrt(var): ACT sqrt + one Newton step  s1 = 0.5*(s0 + var/s0)
    nc.scalar.activation(s0[:], var[:], AF.Sqrt)
    nc.vector.reciprocal(tmp[:], s0[:])
    nc.vector.tensor_mul(tmp[:], tmp[:], var[:])
    nc.vector.tensor_add(tmp[:], tmp[:], s0[:])
    nc.vector.tensor_scalar(tmp[:], tmp[:], 0.5, EPS, OP.mult, OP.add)
    nc.vector.reciprocal(tmp[:], tmp[:])                 # 1/(std+eps)
    nc.vector.tensor_scalar_mul(tc_[:], tmp[:], float(alpha))
    nc.vector.tensor_mul(tmp[:], mu[:], tc_[:])
    nc.vector.tensor_scalar(uc_[:], tmp[:], -1.0, float(beta), OP.mult, OP.add)
    if apply_on_scalar:
        nc.scalar.activation(out_ap, x_ap, AF.Identity, bias=uc_[:],
                             scale=tc_[:])
    else:
        nc.vector.tensor_scalar(out_ap, x_ap, tc_[:], uc_[:], OP.mult, OP.add)


def build_program(ln1a, ln1b, ln2a, ln2b, mask_all_ones):
    nc = bacc.Bacc("TRN2", target_bir_lowering=False, debug=False)

    x_d = nc.dram_tensor("x", (S, D), F32, kind="ExternalInput").ap()
    wqT_d = nc.dram_tensor("wqT", (D, D), BF, kind="ExternalInput").ap()
    woT8_d = nc.dram_tensor("woT8", (D, D), FP8, kind="ExternalInput").ap()
    w1L_d = nc.dram_tensor("w1L", (P, FC, DC, P), BF, kind="ExternalInput").ap()
    w2L_d = nc.dram_tensor("w2L", (P, 2, FC // 2, 2, 512), BF, kind="ExternalInput").ap()
    bq_d = nc.dram_tensor("bq_v", (P, DC), F32, kind="ExternalInput").ap()
    b1_d = nc.dram_tensor("b1_v", (P, FC), F32, kind="ExternalInput").ap()
    bo_d = nc.dram_tensor("bo_rep", (P, D), F32, kind="ExternalInput").ap()
    b2_d = nc.dram_tensor("b2_rep", (P, D), F32, kind="ExternalInput").ap()
    if not mask_all_ones:
        m01_d = nc.dram_tensor("m01_v", (P, SC), F32, kind="ExternalInput").ap()
    out_d = nc.dram_tensor("out", (S, D), F32, kind="ExternalOutput").ap()

    x_r = x_d.rearrange("(sc p) d -> sc p d", p=P)
    wqT_r = wqT_d.rearrange("(kc p) o -> kc p o", p=P)
    woT8_r = woT8_d.rearrange("(oc p) d -> oc p d", p=P)
    out_r = out_d.rearrange("(sc p) d -> sc p d", p=P)

    with tile.TileContext(nc) as tc, ExitStack() as st:
        arena = st.enter_context(tc.tile_pool(name="arena", bufs=1))
        small = st.enter_context(tc.tile_pool(name="small", bufs=1))

        # ---- constants ----
        ident_b = small.tile([P, P], BF, name="ident_b")
        make_identity(nc, ident_b[:])
        ident8 = small.tile([P, P], FP8, name="ident8")
        make_identity(nc, ident8[:])
        ebias = small.tile([P, 1], F32, name="ebias")
        nc.gpsimd.memset(ebias[:], EXP_SHIFT)
        bq_sb = small.tile([P, DC], F32, name="bq_sb")
        nc.sync.dma_start(bq_sb[:], bq_d)
        b1_sb = small.tile([P, FC], F32, name="b1_sb")
        nc.sync.dma_start(b1_sb[:], b1_d)
        bo_rep = small.tile([P, D], F32, name="bo_rep")
        nc.gpsimd.dma_start(bo_rep[:], bo_d)
        b2_rep = small.tile([P, D], F32, name="b2_rep")
        nc.gpsimd.dma_start(b2_rep[:], b2_d)
        if not mask_all_ones:
            m01_sb = small.tile([P, SC], F32, name="m01_sb")
            nc.sync.dma_start(m01_sb[:], m01_d)

        # ---- persistent sbuf tiles ----
        qT8 = arena.tile([P, DC, S], FP8, tag="qT8", name="qT8")
        qh8e = arena.tile([P, SC, H * HW], FP8, tag="qh8e", name="qh8e")
        ctxN = arena.tile([P, SC, D], FP8, tag="ctxN", name="ctxN")
        res1 = arena.tile([P, SC, D], F32, tag="res1", name="res1")
        n1T = arena.tile([P, DC, S], BF, tag="n1T_ctxT8", name="n1T")
        wq_sb = arena.tile([P, DC, D], BF, tag="wq_n2t", bufs=2, name="wq_sb")
        woT8_sb = arena.tile([P, DC, D], FP8, tag="woT8", name="woT8_sb")

        # zero the Z-ones / pad cols of qh8e
        for h in range(H):
            nc.gpsimd.memset(qh8e[:, :, ds(h * HW + DK, 1)], 1.0)
            nc.gpsimd.memset(qh8e[:, :, ds(h * HW + DK + 1, 1)], 0.0)

        for kc in range(DC):
            (nc.sync if kc % 2 == 0 else nc.gpsimd).dma_start(
                wq_sb[:, kc], wqT_r[kc])
        for oc in range(DC):
            nc.gpsimd.dma_start(woT8_sb[:, oc], woT8_r[oc])

        # =========== phase 1: LN1 streamed per chunk + n1T transposes =======
        with tc.tile_pool(name="ps1", bufs=1, space="PSUM") as ps1:
            for sc in range(SC):
                xts = arena.tile([P, D], F32, tag="xts", bufs=2, name="xts")
                nc.sync.dma_start(xts[:], x_r[sc])
                n1s = arena.tile([P, D], BF, tag="n1s", bufs=2, name="n1s")
                _emit_ln_chunk(nc, small, xts[:], n1s[:], n1s[:], ln1a, ln1b,
                               apply_on_scalar=True)
                for cb in range(DC):
                    tpB = ps1.tile([P, P], BF, tag="tpB", bufs=4, name="tpB")
                    nc.tensor.transpose(tpB[:], n1s[:, ts(cb, P)], ident_b[:])
                    nc.vector.tensor_copy(n1T[:, cb, ts(sc, P)], tpB[:])

        # attention helpers ---------------------------------------------------
        def qproj_head(psQ, ps_tp8, hp):
            """q projection for feature chunk oc=hp -> qT8 + qh8e slices."""
            for b in range(2):
                pb = psQ.tile([P, 512], F32, tag="pb", bufs=1, name="pb")
                for kc in range(DC):
                    nc.tensor.matmul(
                        pb[:], wq_sb[:, kc, ts(hp, P)],
                        n1T[:, kc, ds(512 * b, 512)],
                        start=(kc == 0), stop=(kc == DC - 1),
                    )
                nc.vector.tensor_scalar(
                    qT8[:, hp, ds(512 * b, 512)], pb[:],
                    bq_sb[:, ds(hp, 1)], None, OP.add,
                )
            for sc in range(SC):
                tp8 = ps_tp8.tile([P, P, 2], FP8, tag="tp8", bufs=1,
                                  name="tp8")
                nc.tensor.transpose(
                    tp8[:, :, ds(0, 1)], qT8[:, hp, ts(sc, P)], ident8[:])
                for hl in range(2):
                    nc.vector.tensor_scalar_mul(
                        qh8e[:, sc, ds((2 * hp + hl) * HW, DK)],
                        tp8[:, ds(hl * DK, DK), 0], CTX_SCALE,
                    )

        def attn_head(psS, psC, hp, half):
            """scores+exp+ctx for head pair hp, query half `half`."""
            ec8 = arena.tile([P, SC, 1024], FP8, tag="ec8", bufs=2, name="ec8")
            for c in range(SC):
                sp = psS.tile([P, 1024], F32, tag="sp", bufs=2, name="sp")
                for hl in range(2):
                    lo = hl * DK
                    nc.tensor.matmul(
                        sp[:, ds(hl * 512, 512)],
                        qT8[ds(lo, DK), hp, ts(c, P)],
                        qT8[ds(lo, DK), hp, ds(512 * half, 512)],
                        start=True, stop=True,
                        tile_position=(lo, 0),
                    )
                nc.scalar.activation(
                    ec8[:, c], sp[:], AF.Exp, bias=ebias[:], scale=0.125,
                )
                if not mask_all_ones:
                    nc.vector.tensor_scalar_mul(
                        ec8[:, c], ec8[:, c], m01_sb[:, ds(c, 1)],
                    )
            for hl in range(2):
                h = 2 * hp + hl
                for qc in range(4):
                    cxp = psC.tile([P, 128], F32, tag="cxp", bufs=2,
                                   name="cxp")
                    for ci in range(SC // 2):
                        nc.tensor.matmul(
                            cxp[:, ds(0, HW)],
                            ec8[:, ds(2 * ci, 2),
                                ds(hl * 512 + qc * P, P)],
                            qh8e[:, ds(2 * ci, 2), ds(h * HW, HW)],
                            start=(ci == 0), stop=(ci == SC // 2 - 1),
                            perf_mode=DR,
                        )
                    rz = small.tile([P, 1], F32, tag="rz", bufs=3, name="rz")
                    nc.vector.reciprocal_approx_fast(rz[:], cxp[:, ds(DK, 1)])
                    nc.vector.tensor_scalar(
                        ctxN[:, half * 4 + qc, ds(h * DK, DK)],
                        cxp[:, ds(0, DK)], rz[:], None, OP.mult,
                    )

        def ctx_transpose_half(ps_tp, half, sls=range(4)):
            for sl in sls:
                sc = half * 4 + sl
                for oc in range(DC):
                    tpC8 = ps_tp.tile([P, P, 2], FP8, tag="tpX", bufs=1,
                                      name="tpC8")
                    nc.tensor.transpose(
                        tpC8[:, :, ds(0, 1)], ctxN[:, sc, ts(oc, P)],
                        ident8[:])
                    nc.vector.tensor_copy(
                        ctxT8_t[:, oc, ts(sc, P)], tpC8[:, :, 0])

        def wo_half(psW, half, sls=range(4)):
            for sl in sls:
                sc = half * 4 + sl
                xre = arena.tile([P, D], F32, tag="xre", bufs=1, name="xre")
                nc.sync.dma_start(xre[:], x_r[sc])
                for dh in range(2):
                    wp = psW.tile([P, 512], F32, tag="wp", bufs=1, name="wp")
                    for oi in range(DC // 2):
                        nc.tensor.matmul(
                            wp[:],
                            ctxT8_t[:, ds(2 * oi, 2), ts(sc, P)],
                            woT8_sb[:, ds(2 * oi, 2), ds(512 * dh, 512)],
                            start=(oi == 0), stop=(oi == DC // 2 - 1),
                            perf_mode=DR,
                        )
                    nc.vector.scalar_tensor_tensor(
                        res1[:, sc, ds(512 * dh, 512)], wp[:], OUT_SCALE,
                        xre[:, ds(512 * dh, 512)], OP.mult, OP.add,
                    )
                    nc.vector.tensor_add(
                        res1[:, sc, ds(512 * dh, 512)],
                        res1[:, sc, ds(512 * dh, 512)],
                        bo_rep[:, ds(512 * dh, 512)],
                    )

        def ln2_half(ps_tp, half, n2Th, sls=range(4)):
            for sl in sls:
                sc = half * 4 + sl
                n2s = arena.tile([P, D], BF, tag="n2s", bufs=2, name="n2s")
                _emit_ln_chunk(nc, small, res1[:, sc], n2s[:], n2s[:],
                               ln2a, ln2b)
                for cb in range(DC):
                    tpC = ps_tp.tile([P, P], BF, tag="tpX", bufs=1,
                                     name="tpC")
                    nc.tensor.transpose(tpC[:], n2s[:, ts(cb, P)], ident_b[:])
                    nc.vector.tensor_copy(n2Th[:, cb, ts(sl, P)], tpC[:])
                nc.vector.tensor_add(res1[:, sc], res1[:, sc], b2_rep[:])

        def ffn1_chunk(psF, wsp, n2Th, h1t, lfc, fc, relu_on_scalar):
            wts = wsp.tile([P, DC, P], BF, tag="w1s", bufs=3, name="w1s")
            (nc.sync if fc % 2 == 0 else nc.gpsimd).dma_start(
                wts[:], w1L_d[:, fc])
            fp = psF.tile([P, 512], F32, tag="f1ps", bufs=2, name="f1ps")
            for dc in range(DC):
                nc.tensor.matmul(
                    fp[:], wts[:, dc], n2Th[:, dc, :],
                    start=(dc == 0), stop=(dc == DC - 1),
                )
            if relu_on_scalar:
                nc.scalar.activation(
                    h1t[:, lfc], fp[:], AF.Relu, bias=b1_sb[:, ds(fc, 1)],
                )
            else:
                nc.vector.tensor_scalar(
                    h1t[:, lfc], fp[:], b1_sb[:, ds(fc, 1)], 0.0,
                    OP.add, OP.max,
                )

        def ffn2_drain(half, dh, ops):
            for sl in range(4):
                sc = half * 4 + sl
                nc.vector.tensor_add(
                    res1[:, sc, ds(512 * dh, 512)], ops[sl][:],
                    res1[:, sc, ds(512 * dh, 512)],
                )
                (nc.gpsimd if sl % 2 == 0 else nc.scalar).dma_start(
                    out_r[sc][:, ds(512 * dh, 512)],
                    res1[:, sc, ds(512 * dh, 512)],
                )

        def ffn2_mms(ops, h1at, w2t, fc2):
            for fi in range(2):
                fc = 2 * fc2 + fi
                h1t, lfc = h1at(fc)
                for sl in range(4):
                    nc.tensor.matmul(
                        ops[sl][:], h1t[:, lfc, ts(sl, P)], w2t[:, fi],
                        start=(fc == 0), stop=(fc == FC - 1),
                    )

        # ================== phase 2: qproj pipeline + attention half 0 ======
        ctxT8_t = None
        with tc.tile_pool(name="wstream", bufs=1) as wsp:
            with tc.tile_pool(name="psSp", bufs=1, space="PSUM") as psS, \
                 tc.tile_pool(name="psCx", bufs=1, space="PSUM") as psC:
                with tc.tile_pool(name="psQ", bufs=1, space="PSUM") as psQ:
                    for hp in range(DC):
                        qproj_head(psQ, psQ, hp)
                        if hp >= 1:
                            attn_head(psS, psC, hp - 1, 0)
                    attn_head(psS, psC, DC - 1, 0)

                # ========= phase 3: ctxT8(0), wo(0), ln2(0) =================
                ctxT8_t = arena.tile([P, DC, S], FP8, tag="n1T_ctxT8",
                                     name="ctxT8")
                n2Th0 = arena.tile([P, DC, 512], BF, tag="wq_n2t",
                                   bufs=2, name="n2Th0")
                with tc.tile_pool(name="ps3", bufs=1, space="PSUM") as ps3:
                    ctx_transpose_half(ps3, 0)
                    wo_half(ps3, 0)
                    ln2_half(ps3, 0, n2Th0)

                # ==== phase 4: attention half 1 with ffn1(0) interleaved ====
                h1a = arena.tile([P, FC, 512], BF, tag="xt_h1", name="h1a")
                with tc.tile_pool(name="ps4", bufs=1, space="PSUM") as ps4:
                    for hp in range(DC):
                        attn_head(psS, psC, hp, 1)
                        for fc in range(4 * hp, 4 * hp + 4):
                            ffn1_chunk(ps4, wsp, n2Th0, h1a, fc, fc, False)

            # ==== phase 5 + pass A: ctxT8(1)/wo(1)/ln2(1) and ffn1(1),
            # with ffn2(half0, dh0) matmuls injected as PE filler ============
            n2Th1 = arena.tile([P, DC, 512], BF, tag="wq_n2t", bufs=2,
                               name="n2Th1")
            h1b_parts = [
                arena.tile([P, 8, 512], BF, tag="ec8", bufs=2, name="h1b0"),
                arena.tile([P, 8, 512], BF, tag="ec8", bufs=2, name="h1b1"),
                arena.tile([P, 8, 512], BF, tag="qT8", name="h1b2"),
                arena.tile([P, 8, 512], BF, tag="qh8e", name="h1b3"),
            ]
            h1a_at = lambda fc: (h1a, fc)
            h1b_at = lambda fc: (h1b_parts[fc // 8], fc % 8)

            with tc.tile_pool(name="psT6", bufs=1, space="PSUM") as psT6:
                opsA = [psT6.tile([P, 512], F32, tag="f2psA", bufs=4,
                                  name="f2psA") for _ in range(4)]

                def ffn2A_chunk(fc2):
                    w2t = wsp.tile([P, 2, 512], BF, tag="w2s", bufs=3,
                                   name="w2s")
                    (nc.sync if fc2 % 2 == 0 else nc.scalar).dma_start(
                        w2t[:], w2L_d[:, 0, fc2])
                    ffn2_mms(opsA, h1a_at, w2t, fc2)

                with tc.tile_pool(name="ps5", bufs=1, space="PSUM") as ps5:
                    for sl in range(4):
                        ctx_transpose_half(ps5, 1, [sl])
                        wo_half(ps5, 1, [sl])
                        ln2_half(ps5, 1, n2Th1, [sl])
                        ffn2A_chunk(2 * sl)
                        ffn2A_chunk(2 * sl + 1)
                with tc.tile_pool(name="psA6", bufs=1, space="PSUM") as psA6:
                    for fc2 in range(8, FC // 2):
                        ffn2A_chunk(fc2)
                        for fc in range(4 * (fc2 - 8), 4 * (fc2 - 8) + 4):
                            h1t, lfc = h1b_at(fc)
                            ffn1_chunk(psA6, wsp, n2Th1, h1t, lfc, fc, True)
                ffn2_drain(0, 0, opsA)

            # pass B: ffn2(half0, dh1) + ffn2(half1, dh1), shared w2 stream
            with tc.tile_pool(name="psB6", bufs=1, space="PSUM") as psB6:
                opsB0 = [psB6.tile([P, 512], F32, tag="f2psB0", bufs=4,
                                   name="f2psB0") for _ in range(4)]
                opsB1 = [psB6.tile([P, 512], F32, tag="f2psB1", bufs=4,
                                   name="f2psB1") for _ in range(4)]
                for fc2 in range(FC // 2):
                    w2t = wsp.tile([P, 2, 512], BF, tag="w2s", bufs=3,
                                   name="w2s")
                    (nc.sync if fc2 % 2 == 0 else nc.scalar).dma_start(
                        w2t[:], w2L_d[:, 1, fc2])
                    ffn2_mms(opsB0, h1a_at, w2t, fc2)
                    ffn2_mms(opsB1, h1b_at, w2t, fc2)
                ffn2_drain(0, 1, opsB0)
                ffn2_drain(1, 1, opsB1)

            # pass C: ffn2(half1, dh0)
            with tc.tile_pool(name="psC6", bufs=1, space="PSUM") as psC6:
                opsC = [psC6.tile([P, 512], F32, tag="f2psC", bufs=4,
                                  name="f2psC") for _ in range(4)]
                for fc2 in range(FC // 2):
                    w2t = wsp.tile([P, 2, 512], BF, tag="w2s", bufs=3,
                                   name="w2s")
                    (nc.sync if fc2 % 2 == 0 else nc.scalar).dma_start(
                        w2t[:], w2L_d[:, 0, fc2])
                    ffn2_mms(opsC, h1b_at, w2t, fc2)
                ffn2_drain(1, 0, opsC)

    nc.compile()
    return nc


def _prep_inputs(inputs):
    f32 = lambda a: np.ascontiguousarray(np.asarray(a, dtype=np.float32))
    bfT = lambda a: np.ascontiguousarray(
        np.asarray(a, dtype=np.float32).T.astype(ml_dtypes.bfloat16))
    x = f32(inputs["x"])                      # [B, S, D]
    mask = np.asarray(inputs["src_mask"])     # [B, 1, 1, S] int32
    wqT = bfT(inputs["wq"])                   # [D, D] (in, out)
    woT8 = np.ascontiguousarray(
        (np.asarray(inputs["wo"], dtype=np.float32).T * WO_SCALE)
        .astype(ml_dtypes.float8_e4m3))
    w1 = np.asarray(inputs["w1"], dtype=np.float32)      # [DFF, D]
    w2 = np.asarray(inputs["w2"], dtype=np.float32)      # [D, DFF]
    # w1L[p, fc, dc, f] = w1[fc*128+f, dc*128+p]; 2KB-contiguous DMA chunks
    w1L = np.ascontiguousarray(
        w1.reshape(FC, P, DC, P).transpose(3, 0, 2, 1)
        .astype(ml_dtypes.bfloat16))
    # w2L[p, dh, fc2, i, d] = w2[dh*512+d, (2*fc2+i)*128+p]
    w2L = np.ascontiguousarray(
        w2.reshape(2, 512, FC // 2, 2, P).transpose(4, 0, 2, 3, 1)
        .astype(ml_dtypes.bfloat16))
    bq_v = np.ascontiguousarray(f32(inputs["bq"]).reshape(DC, P).T)
    b1_v = np.ascontiguousarray(f32(inputs["b1"]).reshape(FC, P).T)
    bo_rep = np.ascontiguousarray(np.tile(f32(inputs["bo"]), (P, 1)))
    b2_rep = np.ascontiguousarray(np.tile(f32(inputs["b2"]), (P, 1)))
    scal = lambda k: float(np.asarray(inputs[k]).reshape(-1)[0])
    ln = (scal("ln1_a"), scal("ln1_b"), scal("ln2_a"), scal("ln2_b"))
    mask_all_ones = bool((mask != 0).all())

    shared = dict(wqT=wqT, woT8=woT8, w1L=w1L, w2L=w2L, bq_v=bq_v, b1_v=b1_v,
                  bo_rep=bo_rep, b2_rep=b2_rep)
    in_maps = []
    for b in range(NB):
        m = dict(shared)
        m["x"] = np.ascontiguousarray(x[b])
        if not mask_all_ones:
            m01 = (mask[b].reshape(S) != 0).astype(np.float32)
            m["m01_v"] = np.ascontiguousarray(m01.reshape(SC, P).T)
        in_maps.append(m)
    return in_maps, ln, mask_all_ones


last_nc = None
last_in_maps = None


def kernel(**inputs):
    global last_nc, last_in_maps
    in_maps, ln, mask_all_ones = _prep_inputs(inputs)
    nc = build_program(*ln, mask_all_ones)
    last_nc, last_in_maps = nc, in_maps
    res = bass_utils.run_bass_kernel_spmd(
        nc, in_maps, core_ids=list(range(NB)), trace=False,
    )
    out = np.stack([np.asarray(res.results[b]["out"]) for b in range(NB)])
    return out.astype(np.float32)


# revision 16
# speedup vs baseline: 1.0336x; 1.0336x over previous
"""Trainium2 Bass kernel for nn_EncoderBlock (dense transformer encoder block).

Data parallel: batch B=8 across 8 NeuronCores, one element per core.

v2 design vs v1 baseline (685us):
  - scores matmuls in fp8 DoubleRow (zero-padded K=64 subtile): 2x.
  - ctx computed "flipped" (out = [queries, feats]) in fp8 DoubleRow with a
    fused ones-column producing the softmax normalizer Z in the same psum
    tile; per-partition normalize via reciprocal_approx_fast + tensor_scalar.
    Kills the ones-matmul for Z and the replicated [128,512] reciprocal.
  - wo in fp8 DoubleRow; wo weights pre-scaled x64 host-side and ctx scaled
    x16 on-chip (fp8 subnormal avoidance), un-scaled by 1/1024 in the
    residual add.
  - qproj per head-pair pipelined under the attention-half-0 exp stream;
    ffn1(half 0) interleaved under the attention-half-1 exp stream.
  - LN sum/sum-of-squares both via ScalarE ACT accumulate (Copy / Square).
FFN and qproj matmuls stay bf16 (fp8 there would blow the 2e-2 error gate).
"""

import sys

sys.path.insert(0, "/opt/trn_rl_repo")

import numpy as np
import ml_dtypes
from contextlib import ExitStack

import concourse.bass as bass
import concourse.tile as tile
from concourse import bacc, mybir
from concourse import bass_utils
from concourse.bass import ts, ds
from concourse.masks import make_identity

BF = mybir.dt.bfloat16
F32 = mybir.dt.float32
FP8 = mybir.dt.float8e4
AF = mybir.ActivationFunctionType
OP = mybir.AluOpType
DR = mybir.MatmulPerfMode.DoubleRow

P = 128
S = 1024          # sequence length per core
D = 1024          # d_model
H = 16            # heads
DK = 64           # head dim
DFF = 4096
NB = 8            # batch = number of cores
SC = S // P       # 8 sequence chunks
DC = D // P       # 8 feature chunks
FC = DFF // P     # 32 ff chunks
EPS = 1e-6
EXP_SHIFT = -2.0   # constant shift inside exp; cancels in softmax ratio
CTX_SCALE = 16.0   # on-chip scale of v (=q) into qh8e; keeps ctx out of
WO_SCALE = 64.0    # fp8 subnormal range.  woT8 = fp8(64*wo) host-side.
OUT_SCALE = 1.0 / (CTX_SCALE * WO_SCALE)
HW = 66            # per-head stride in qh8e: 64 feats + Z-ones col + pad

last_exec_time_ns = None


def _emit_ln_chunk(nc, small, x_ap, out_ap, scratch_ap, alpha, beta,
                   apply_on_scalar=False):
    """Bessel-corrected LN of one [P, D] chunk, stats per token on partitions.
    n = (x - mu)/(std + eps)*alpha + beta.  Sum and sum-of-squares come from
    ScalarE ACT accumulate (Copy / Square) writing scratch_ap; scratch is
    overwritten by the final apply (out_ap may alias scratch_ap)."""
    s1 = small.tile([P, 1], F32, tag="ln_s1", bufs=3, name="ln_s1")
    sq = small.tile([P, 1], F32, tag="ln_sq", bufs=3, name="ln_sq")
    mu = small.tile([P, 1], F32, tag="ln_mu", bufs=3, name="ln_mu")
    var = small.tile([P, 1], F32, tag="ln_var", bufs=3, name="ln_var")
    tmp = small.tile([P, 1], F32, tag="ln_tmp", bufs=3, name="ln_tmp")
    s0 = small.tile([P, 1], F32, tag="ln_s0", bufs=3, name="ln_s0")
    tc_ = small.tile([P, 1], F32, tag="ln_tc", bufs=3, name="ln_tc")
    uc_ = small.tile([P, 1], F32, tag="ln_uc", bufs=3, name="ln_uc")

    nc.scalar.activation(scratch_ap, x_ap, AF.Copy, accum_out=s1[:])
    nc.scalar.activation(scratch_ap, x_ap, AF.Square, accum_out=sq[:])
    nc.vector.tensor_scalar_mul(mu[:], s1[:], 1.0 / D)
    nc.vector.tensor_mul(tmp[:], mu[:], mu[:])
    nc.vector.tensor_scalar_mul(var[:], sq[:], 1.0 / (D - 1))
    nc.vector.tensor_scalar_mul(tmp[:], tmp[:], float(D) / (D - 1))
    nc.vector.tensor_sub(var[:], var[:], tmp[:])
    # std = sqrt(var): ACT sqrt + one Newton step  s1 = 0.5*(s0 + var/s0)
    nc.scalar.activation(s0[:], var[:], AF.Sqrt)
    nc.vector.reciprocal(tmp[:], s0[:])
    nc.vector.tensor_mul(tmp[:], tmp[:], var[:])
    nc.vector.tensor_add(tmp[:], tmp[:], s0[:])
    nc.vector.tensor_scalar(tmp[:], tmp[:], 0.5, EPS, OP.mult, OP.add)
    nc.vector.reciprocal(tmp[:], tmp[:])                 # 1/(std+eps)
    nc.vector.tensor_scalar_mul(tc_[:], tmp[:], float(alpha))
    nc.vector.tensor_mul(tmp[:], mu[:], tc_[:])
    nc.vector.tensor_scalar(uc_[:], tmp[:], -1.0, float(beta), OP.mult, OP.add)
    if apply_on_scalar:
        nc.scalar.activation(out_ap, x_ap, AF.Identity, bias=uc_[:],
                             scale=tc_[:])
    else:
        nc.vector.tensor_scalar(out_ap, x_ap, tc_[:], uc_[:], OP.mult, OP.add)


def build_program(ln1a, ln1b, ln2a, ln2b, mask_all_ones):
    nc = bacc.Bacc("TRN2", target_bir_lowering=False, debug=False)

    x_d = nc.dram_tensor("x", (S, D), F32, kind="ExternalInput").ap()
    wqT_d = nc.dram_tensor("wqT", (D, D), BF, kind="ExternalInput").ap()
    woT8_d = nc.dram_tensor("woT8", (D, D), FP8, kind="ExternalInput").ap()
    w1L_d = nc.dram_tensor("w1L", (P, FC, DC, P), BF, kind="ExternalInput").ap()
    w2L_d = nc.dram_tensor("w2L", (P, 2, FC // 2, 2, 512), BF, kind="ExternalInput").ap()
    bq_d = nc.dram_tensor("bq_v", (P, DC), F32, kind="ExternalInput").ap()
    b1_d = nc.dram_tensor("b1_v", (P, FC), F32, kind="ExternalInput").ap()
    bo_d = nc.dram_tensor("bo_rep", (P, D), F32, kind="ExternalInput").ap()
    b2_d = nc.dram_tensor("b2_rep", (P, D), F32, kind="ExternalInput").ap()
    if not mask_all_ones:
        m01_d = nc.dram_tensor("m01_v", (P, SC), F32, kind="ExternalInput").ap()
    out_d = nc.dram_tensor("out", (S, D), F32, kind="ExternalOutput").ap()

    x_r = x_d.rearrange("(sc p) d -> sc p d", p=P)
    wqT_r = wqT_d.rearrange("(kc p) o -> kc p o", p=P)
    woT8_r = woT8_d.rearrange("(oc p) d -> oc p d", p=P)
    out_r = out_d.rearrange("(sc p) d -> sc p d", p=P)

    with tile.TileContext(nc) as tc, ExitStack() as st:
        arena = st.enter_context(tc.tile_pool(name="arena", bufs=1))
        small = st.enter_context(tc.tile_pool(name="small", bufs=1))

        # ---- constants ----
        ident_b = small.tile([P, P], BF, name="ident_b")
        make_identity(nc, ident_b[:])
        ident8 = small.tile([P, P], FP8, name="ident8")
        make_identity(nc, ident8[:])
        ebias = small.tile([P, 1], F32, name="ebias")
        nc.gpsimd.memset(ebias[:], EXP_SHIFT)
        bq_sb = small.tile([P, DC], F32, name="bq_sb")
        nc.sync.dma_start(bq_sb[:], bq_d)
        b1_sb = small.tile([P, FC], F32, name="b1_sb")
        nc.sync.dma_start(b1_sb[:], b1_d)
        bo_rep = small.tile([P, D], F32, name="bo_rep")
        nc.gpsimd.dma_start(bo_rep[:], bo_d)
        b2_rep = small.tile([P, D], F32, name="b2_rep")
        nc.gpsimd.dma_start(b2_rep[:], b2_d)
        if not mask_all_ones:
            m01_sb = small.tile([P, SC], F32, name="m01_sb")
            nc.sync.dma_start(m01_sb[:], m01_d)

        # ---- persistent sbuf tiles ----
        qT8 = arena.tile([P, DC, S], FP8, tag="qT8", name="qT8")
        qh8e = arena.tile([P, SC, H * HW], FP8, tag="qh8e", name="qh8e")
        ctxN = arena.tile([P, SC, D], FP8, tag="ctxN", name="ctxN")
        res1 = arena.tile([P, SC, D], F32, tag="res1", name="res1")
        n1T = arena.tile([P, DC, S], BF, tag="n1T_ctxT8", name="n1T")
        wq_sb = arena.tile([P, DC, D], BF, tag="wq_n2t", bufs=2, name="wq_sb")
        woT8_sb = arena.tile([P, DC, D], FP8, tag="woT8", name="woT8_sb")

        # zero the Z-ones / pad cols of qh8e
        for h in range(H):
            nc.gpsimd.memset(qh8e[:, :, ds(h * HW + DK, 1)], 1.0)
            nc.gpsimd.memset(qh8e[:, :, ds(h * HW + DK + 1, 1)], 0.0)

        for kc in range(DC):
            (nc.sync if kc % 2 == 0 else nc.gpsimd).dma_start(
                wq_sb[:, kc], wqT_r[kc])
        for oc in range(DC):
            nc.gpsimd.dma_start(woT8_sb[:, oc], woT8_r[oc])

        # =========== phase 1: LN1 streamed per chunk + n1T transposes =======
        with tc.tile_pool(name="ps1", bufs=1, space="PSUM") as ps1:
            for sc in range(SC):
                xts = arena.tile([P, D], F32, tag="xts", bufs=2, name="xts")
                nc.sync.dma_start(xts[:], x_r[sc])
                n1s = arena.tile([P, D], BF, tag="n1s", bufs=2, name="n1s")
                _emit_ln_chunk(nc, small, xts[:], n1s[:], n1s[:], ln1a, ln1b)
                for cb in range(DC):
                    tpB = ps1.tile([P, P], BF, tag="tpB", bufs=4, name="tpB")
                    nc.tensor.transpose(tpB[:], n1s[:, ts(cb, P)], ident_b[:])
                    nc.vector.tensor_copy(n1T[:, cb, ts(sc, P)], tpB[:])

        # attention helpers ---------------------------------------------------
        def qproj_head(psQ, ps_tp8, hp):
            """q projection for feature chunk oc=hp -> qT8 + qh8e slices."""
            for b in range(2):
                pb = psQ.tile([P, 512], F32, tag="pb", bufs=1, name="pb")
                for kc in range(DC):
                    nc.tensor.matmul(
                        pb[:], wq_sb[:, kc, ts(hp, P)],
                        n1T[:, kc, ds(512 * b, 512)],
                        start=(kc == 0), stop=(kc == DC - 1),
                    )
                nc.vector.tensor_scalar(
                    qT8[:, hp, ds(512 * b, 512)], pb[:],
                    bq_sb[:, ds(hp, 1)], None, OP.add,
                )
            for sc in range(SC):
                tp8 = ps_tp8.tile([P, P, 2], FP8, tag="tp8", bufs=1,
                                  name="tp8")
                nc.tensor.transpose(
                    tp8[:, :, ds(0, 1)], qT8[:, hp, ts(sc, P)], ident8[:])
                for hl in range(2):
                    nc.vector.tensor_scalar_mul(
                        qh8e[:, sc, ds((2 * hp + hl) * HW, DK)],
                        tp8[:, ds(hl * DK, DK), 0], CTX_SCALE,
                    )

        def attn_head(psS, psC, hp, half):
            """scores+exp+ctx for head pair hp, query half `half`."""
            ec8 = arena.tile([P, SC, 1024], FP8, tag="ec8", bufs=2, name="ec8")
            for c in range(SC):
                sp = psS.tile([P, 1024], F32, tag="sp", bufs=2, name="sp")
                for hl in range(2):
                    lo = hl * DK
                    nc.tensor.matmul(
                        sp[:, ds(hl * 512, 512)],
                        qT8[ds(lo, DK), hp, ts(c, P)],
                        qT8[ds(lo, DK), hp, ds(512 * half, 512)],
                        start=True, stop=True,
                        tile_position=(lo, 0),
                    )
                nc.scalar.activation(
                    ec8[:, c], sp[:], AF.Exp, bias=ebias[:], scale=0.125,
                )
                if not mask_all_ones:
                    nc.vector.tensor_scalar_mul(
                        ec8[:, c], ec8[:, c], m01_sb[:, ds(c, 1)],
                    )
            for hl in range(2):
                h = 2 * hp + hl
                for qc in range(4):
                    cxp = psC.tile([P, 128], F32, tag="cxp", bufs=2,
                                   name="cxp")
                    for ci in range(SC // 2):
                        nc.tensor.matmul(
                            cxp[:, ds(0, HW)],
                            ec8[:, ds(2 * ci, 2),
                                ds(hl * 512 + qc * P, P)],
                            qh8e[:, ds(2 * ci, 2), ds(h * HW, HW)],
                            start=(ci == 0), stop=(ci == SC // 2 - 1),
                            perf_mode=DR,
                        )
                    rz = small.tile([P, 1], F32, tag="rz", bufs=3, name="rz")
                    nc.vector.reciprocal_approx_fast(rz[:], cxp[:, ds(DK, 1)])
                    nc.vector.tensor_scalar(
                        ctxN[:, half * 4 + qc, ds(h * DK, DK)],
                        cxp[:, ds(0, DK)], rz[:], None, OP.mult,
                    )

        def ctx_transpose_half(ps_tp, half, sls=range(4)):
            for sl in sls:
                sc = half * 4 + sl
                for oc in range(DC):
                    tpC8 = ps_tp.tile([P, P, 2], FP8, tag="tpX", bufs=1,
                                      name="tpC8")
                    nc.tensor.transpose(
                        tpC8[:, :, ds(0, 1)], ctxN[:, sc, ts(oc, P)],
                        ident8[:])
                    nc.vector.tensor_copy(
                        ctxT8_t[:, oc, ts(sc, P)], tpC8[:, :, 0])

        def wo_half(psW, half, sls=range(4)):
            for sl in sls:
                sc = half * 4 + sl
                xre = arena.tile([P, D], F32, tag="xre", bufs=1, name="xre")
                nc.sync.dma_start(xre[:], x_r[sc])
                for dh in range(2):
                    wp = psW.tile([P, 512], F32, tag="wp", bufs=1, name="wp")
                    for oi in range(DC // 2):
                        nc.tensor.matmul(
                            wp[:],
                            ctxT8_t[:, ds(2 * oi, 2), ts(sc, P)],
                            woT8_sb[:, ds(2 * oi, 2), ds(512 * dh, 512)],
                            start=(oi == 0), stop=(oi == DC // 2 - 1),
                            perf_mode=DR,
                        )
                    nc.vector.scalar_tensor_tensor(
                        res1[:, sc, ds(512 * dh, 512)], wp[:], OUT_SCALE,
                        xre[:, ds(512 * dh, 512)], OP.mult, OP.add,
                    )
                    nc.vector.tensor_add(
                        res1[:, sc, ds(512 * dh, 512)],
                        res1[:, sc, ds(512 * dh, 512)],
                        bo_rep[:, ds(512 * dh, 512)],
                    )

        def ln2_half(ps_tp, half, n2Th, sls=range(4)):
            for sl in sls:
                sc = half * 4 + sl
                n2s = arena.tile([P, D], BF, tag="n2s", bufs=2, name="n2s")
                _emit_ln_chunk(nc, small, res1[:, sc], n2s[:], n2s[:],
                               ln2a, ln2b)
                for cb in range(DC):
                    tpC = ps_tp.tile([P, P], BF, tag="tpX", bufs=1,
                                     name="tpC")
                    nc.tensor.transpose(tpC[:], n2s[:, ts(cb, P)], ident_b[:])
                    nc.vector.tensor_copy(n2Th[:, cb, ts(sl, P)], tpC[:])
                nc.vector.tensor_add(res1[:, sc], res1[:, sc], b2_rep[:])

        def ffn1_chunk(psF, wsp, n2Th, h1t, lfc, fc, relu_on_scalar):
            wts = wsp.tile([P, DC, P], BF, tag="w1s", bufs=3, name="w1s")
            (nc.sync if fc % 2 == 0 else nc.gpsimd).dma_start(
                wts[:], w1L_d[:, fc])
            fp = psF.tile([P, 512], F32, tag="f1ps", bufs=2, name="f1ps")
            for dc in range(DC):
                nc.tensor.matmul(
                    fp[:], wts[:, dc], n2Th[:, dc, :],
                    start=(dc == 0), stop=(dc == DC - 1),
                )
            if relu_on_scalar:
                nc.scalar.activation(
                    h1t[:, lfc], fp[:], AF.Relu, bias=b1_sb[:, ds(fc, 1)],
                )
            else:
                nc.vector.tensor_scalar(
                    h1t[:, lfc], fp[:], b1_sb[:, ds(fc, 1)], 0.0,
                    OP.add, OP.max,
                )

        def ffn2_drain(half, dh, ops):
            for sl in range(4):
                sc = half * 4 + sl
                nc.vector.tensor_add(
                    res1[:, sc, ds(512 * dh, 512)], ops[sl][:],
                    res1[:, sc, ds(512 * dh, 512)],
                )
                (nc.gpsimd if sl % 2 == 0 else nc.scalar).dma_start(
                    out_r[sc][:, ds(512 * dh, 512)],
                    res1[:, sc, ds(512 * dh, 512)],
                )

        def ffn2_mms(ops, h1at, w2t, fc2):
            for fi in range(2):
                fc = 2 * fc2 + fi
                h1t, lfc = h1at(fc)
                for sl in range(4):
                    nc.tensor.matmul(
                        ops[sl][:], h1t[:, lfc, ts(sl, P)], w2t[:, fi],
                        start=(fc == 0), stop=(fc == FC - 1),
                    )

        # ================== phase 2: qproj pipeline + attention half 0 ======
        ctxT8_t = None
        with tc.tile_pool(name="wstream", bufs=1) as wsp:
            with tc.tile_pool(name="psSp", bufs=1, space="PSUM") as psS, \
                 tc.tile_pool(name="psCx", bufs=1, space="PSUM") as psC:
                with tc.tile_pool(name="psQ", bufs=1, space="PSUM") as psQ:
                    for hp in range(DC):
                        qproj_head(psQ, psQ, hp)
                        if hp >= 1:
                            attn_head(psS, psC, hp - 1, 0)
                    attn_head(psS, psC, DC - 1, 0)

                # ========= phase 3: ctxT8(0), wo(0), ln2(0) =================
                ctxT8_t = arena.tile([P, DC, S], FP8, tag="n1T_ctxT8",
                                     name="ctxT8")
                n2Th0 = arena.tile([P, DC, 512], BF, tag="wq_n2t",
                                   bufs=2, name="n2Th0")
                with tc.tile_pool(name="ps3", bufs=1, space="PSUM") as ps3:
                    ctx_transpose_half(ps3, 0)
                    wo_half(ps3, 0)
                    ln2_half(ps3, 0, n2Th0)

                # ==== phase 4: attention half 1 with ffn1(0) interleaved ====
                h1a = arena.tile([P, FC, 512], BF, tag="xt_h1", name="h1a")
                with tc.tile_pool(name="ps4", bufs=1, space="PSUM") as ps4:
                    for hp in range(DC):
                        attn_head(psS, psC, hp, 1)
                        for fc in range(4 * hp, 4 * hp + 4):
                            ffn1_chunk(ps4, wsp, n2Th0, h1a, fc, fc, False)

            # ==== phase 5 + pass A: ctxT8(1)/wo(1)/ln2(1) and ffn1(1),
            # with ffn2(half0, dh0) matmuls injected as PE filler ============
            n2Th1 = arena.tile([P, DC, 512], BF, tag="wq_n2t", bufs=2,
                               name="n2Th1")
            h1b_parts = [
                arena.tile([P, 8, 512], BF, tag="ec8", bufs=2, name="h1b0"),
                arena.tile([P, 8, 512], BF, tag="ec8", bufs=2, name="h1b1"),
                arena.tile([P, 8, 512], BF, tag="qT8", name="h1b2"),
                arena.tile([P, 8, 512], BF, tag="qh8e", name="h1b3"),
            ]
            h1a_at = lambda fc: (h1a, fc)
            h1b_at = lambda fc: (h1b_parts[fc // 8], fc % 8)

            with tc.tile_pool(name="psT6", bufs=1, space="PSUM") as psT6:
                opsA = [psT6.tile([P, 512], F32, tag="f2psA", bufs=4,
                                  name="f2psA") for _ in range(4)]

                def ffn2A_chunk(fc2):
                    w2t = wsp.tile([P, 2, 512], BF, tag="w2s", bufs=3,
                                   name="w2s")
                    (nc.sync if fc2 % 2 == 0 else nc.scalar).dma_start(
                        w2t[:], w2L_d[:, 0, fc2])
                    ffn2_mms(opsA, h1a_at, w2t, fc2)

                with tc.tile_pool(name="ps5", bufs=1, space="PSUM") as ps5:
                    for sl in range(4):
                        ctx_transpose_half(ps5, 1, [sl])
                        wo_half(ps5, 1, [sl])
                        ln2_half(ps5, 1, n2Th1, [sl])
                        ffn2A_chunk(2 * sl)
                        ffn2A_chunk(2 * sl + 1)
                with tc.tile_pool(name="psA6", bufs=1, space="PSUM") as psA6:
                    for fc2 in range(8, FC // 2):
                        ffn2A_chunk(fc2)
                        for fc in range(4 * (fc2 - 8), 4 * (fc2 - 8) + 4):
                            h1t, lfc = h1b_at(fc)
                            ffn1_chunk(psA6, wsp, n2Th1, h1t, lfc, fc, True)
                ffn2_drain(0, 0, opsA)

            # pass B: ffn2(half0, dh1) + ffn2(half1, dh1), shared w2 stream
            with tc.tile_pool(name="psB6", bufs=1, space="PSUM") as psB6:
                opsB0 = [psB6.tile([P, 512], F32, tag="f2psB0", bufs=4,
                                   name="f2psB0") for _ in range(4)]
                opsB1 = [psB6.tile([P, 512], F32, tag="f2psB1", bufs=4,
                                   name="f2psB1") for _ in range(4)]
                for fc2 in range(FC // 2):
                    w2t = wsp.tile([P, 2, 512], BF, tag="w2s", bufs=3,
                                   name="w2s")
                    (nc.sync if fc2 % 2 == 0 else nc.scalar).dma_start(
                        w2t[:], w2L_d[:, 1, fc2])
                    ffn2_mms(opsB0, h1a_at, w2t, fc2)
                    ffn2_mms(opsB1, h1b_at, w2t, fc2)
                ffn2_drain(0, 1, opsB0)
                ffn2_drain(1, 1, opsB1)

            # pass C: ffn2(half1, dh0)
            with tc.tile_pool(name="psC6", bufs=1, space="PSUM") as psC6:
                opsC = [psC6.tile([P, 512], F32, tag="f2psC", bufs=4,
                                  name="f2psC") for _ in range(4)]
                for fc2 in range(FC // 2):
                    w2t = wsp.tile([P, 2, 512], BF, tag="w2s", bufs=3,
                                   name="w2s")
                    (nc.sync if fc2 % 2 == 0 else nc.scalar).dma_start(
                        w2t[:], w2L_d[:, 0, fc2])
                    ffn2_mms(opsC, h1b_at, w2t, fc2)
                ffn2_drain(1, 0, opsC)

    nc.compile()
    return nc


def _prep_inputs(inputs):
    f32 = lambda a: np.ascontiguousarray(np.asarray(a, dtype=np.float32))
    bfT = lambda a: np.ascontiguousarray(
        np.asarray(a, dtype=np.float32).T.astype(ml_dtypes.bfloat16))
    x = f32(inputs["x"])                      # [B, S, D]
    mask = np.asarray(inputs["src_mask"])     # [B, 1, 1, S] int32
    wqT = bfT(inputs["wq"])                   # [D, D] (in, out)
    woT8 = np.ascontiguousarray(
        (np.asarray(inputs["wo"], dtype=np.float32).T * WO_SCALE)
        .astype(ml_dtypes.float8_e4m3))
    w1 = np.asarray(inputs["w1"], dtype=np.float32)      # [DFF, D]
    w2 = np.asarray(inputs["w2"], dtype=np.float32)      # [D, DFF]
    # w1L[p, fc, dc, f] = w1[fc*128+f, dc*128+p]; 2KB-contiguous DMA chunks
    w1L = np.ascontiguousarray(
        w1.reshape(FC, P, DC, P).transpose(3, 0, 2, 1)
        .astype(ml_dtypes.bfloat16))
    # w2L[p, dh, fc2, i, d] = w2[dh*512+d, (2*fc2+i)*128+p]
    w2L = np.ascontiguousarray(
        w2.reshape(2, 512, FC // 2, 2, P).transpose(4, 0, 2, 3, 1)
        .astype(ml_dtypes.bfloat16))
    bq_v = np.ascontiguousarray(f32(inputs["bq"]).reshape(DC, P).T)
    b1_v = np.ascontiguousarray(f32(inputs["b1"]).reshape(FC, P).T)
    bo_rep = np.ascontiguousarray(np.tile(f32(inputs["bo"]), (P, 1)))
    b2_rep = np.ascontiguousarray(np.tile(f32(inputs["b2"]), (P, 1)))
    scal = lambda k: float(np.asarray(inputs[k]).reshape(-1)[0])
    ln = (scal("ln1_a"), scal("ln1_b"), scal("ln2_a"), scal("ln2_b"))
    mask_all_ones = bool((mask != 0).all())

    shared = dict(wqT=wqT, woT8=woT8, w1L=w1L, w2L=w2L, bq_v=bq_v, b1_v=b1_v,
                  bo_rep=bo_rep, b2_rep=b2_rep)
    in_maps = []
    for b in range(NB):
        m = dict(shared)
        m["x"] = np.ascontiguousarray(x[b])
        if not mask_all_ones:
            m01 = (mask[b].reshape(S) != 0).astype(np.float32)
            m["m01_v"] = np.ascontiguousarray(m01.reshape(SC, P).T)
        in_maps.append(m)
    return in_maps, ln, mask_all_ones


last_nc = None
last_in_maps = None


def kernel(**inputs):
    global last_nc, last_in_maps
    in_maps, ln, mask_all_ones = _prep_inputs(inputs)
    nc = build_program(*ln, mask_all_ones)
    last_nc, last_in_maps = nc, in_maps
    res = bass_utils.run_bass_kernel_spmd(
        nc, in_maps, core_ids=list(range(NB)), trace=False,
    )
    out = np.stack([np.asarray(res.results[b]["out"]) for b in range(NB)])
    return out.astype(np.float32)


# revision 17
# speedup vs baseline: 1.0753x; 1.0403x over previous
"""Trainium2 Bass kernel for nn_EncoderBlock (dense transformer encoder block).

Data parallel: batch B=8 across 8 NeuronCores, one element per core.

v2 design vs v1 baseline (685us):
  - scores matmuls in fp8 DoubleRow (zero-padded K=64 subtile): 2x.
  - ctx computed "flipped" (out = [queries, feats]) in fp8 DoubleRow with a
    fused ones-column producing the softmax normalizer Z in the same psum
    tile; per-partition normalize via reciprocal_approx_fast + tensor_scalar.
    Kills the ones-matmul for Z and the replicated [128,512] reciprocal.
  - wo in fp8 DoubleRow; wo weights pre-scaled x64 host-side and ctx scaled
    x16 on-chip (fp8 subnormal avoidance), un-scaled by 1/1024 in the
    residual add.
  - qproj per head-pair pipelined under the attention-half-0 exp stream;
    ffn1(half 0) interleaved under the attention-half-1 exp stream.
  - LN sum/sum-of-squares both via ScalarE ACT accumulate (Copy / Square).
FFN and qproj matmuls stay bf16 (fp8 there would blow the 2e-2 error gate).
"""

import sys

sys.path.insert(0, "/opt/trn_rl_repo")

import numpy as np
import ml_dtypes
from contextlib import ExitStack

import concourse.bass as bass
import concourse.tile as tile
from concourse import bacc, mybir
from concourse import bass_utils
from concourse.bass import ts, ds
from concourse.masks import make_identity

BF = mybir.dt.bfloat16
F32 = mybir.dt.float32
FP8 = mybir.dt.float8e4
AF = mybir.ActivationFunctionType
OP = mybir.AluOpType
DR = mybir.MatmulPerfMode.DoubleRow

P = 128
S = 1024          # sequence length per core
D = 1024          # d_model
H = 16            # heads
DK = 64           # head dim
DFF = 4096
NB = 8            # batch = number of cores
SC = S // P       # 8 sequence chunks
DC = D // P       # 8 feature chunks
FC = DFF // P     # 32 ff chunks
EPS = 1e-6
EXP_SHIFT = -2.0   # constant shift inside exp; cancels in softmax ratio
CTX_SCALE = 16.0   # on-chip scale of v (=q) into qh8e; keeps ctx out of
WO_SCALE = 64.0    # fp8 subnormal range.  woT8 = fp8(64*wo) host-side.
OUT_SCALE = 1.0 / (CTX_SCALE * WO_SCALE)
HW = 66            # per-head stride in qh8e: 64 feats + Z-ones col + pad

last_exec_time_ns = None


def _emit_ln_chunk(nc, small, x_ap, out_ap, scratch_ap, alpha, beta,
                   apply_on_scalar=False):
    """Bessel-corrected LN of one [P, D] chunk, stats per token on partitions.
    n = (x - mu)/(std + eps)*alpha + beta.  Sum and sum-of-squares come from
    ScalarE ACT accumulate (Copy / Square) writing scratch_ap; scratch is
    overwritten by the final apply (out_ap may alias scratch_ap)."""
    s1 = small.tile([P, 1], F32, tag="ln_s1", bufs=3, name="ln_s1")
    sq = small.tile([P, 1], F32, tag="ln_sq", bufs=3, name="ln_sq")
    mu = small.tile([P, 1], F32, tag="ln_mu", bufs=3, name="ln_mu")
    var = small.tile([P, 1], F32, tag="ln_var", bufs=3, name="ln_var")
    tmp = small.tile([P, 1], F32, tag="ln_tmp", bufs=3, name="ln_tmp")
    s0 = small.tile([P, 1], F32, tag="ln_s0", bufs=3, name="ln_s0")
    tc_ = small.tile([P, 1], F32, tag="ln_tc", bufs=3, name="ln_tc")
    uc_ = small.tile([P, 1], F32, tag="ln_uc", bufs=3, name="ln_uc")

    nc.scalar.activation(scratch_ap, x_ap, AF.Copy, accum_out=s1[:])
    nc.scalar.activation(scratch_ap, x_ap, AF.Square, accum_out=sq[:])
    nc.vector.tensor_scalar_mul(mu[:], s1[:], 1.0 / D)
    nc.vector.tensor_mul(tmp[:], mu[:], mu[:])
    nc.vector.tensor_scalar_mul(var[:], sq[:], 1.0 / (D - 1))
    nc.vector.tensor_scalar_mul(tmp[:], tmp[:], float(D) / (D - 1))
    nc.vector.tensor_sub(var[:], var[:], tmp[:])
    # std = sqrt(var): ACT sqrt + one Newton step  s1 = 0.5*(s0 + var/s0)
    nc.scalar.activation(s0[:], var[:], AF.Sqrt)
    nc.vector.reciprocal(tmp[:], s0[:])
    nc.vector.tensor_mul(tmp[:], tmp[:], var[:])
    nc.vector.tensor_add(tmp[:], tmp[:], s0[:])
    nc.vector.tensor_scalar(tmp[:], tmp[:], 0.5, EPS, OP.mult, OP.add)
    nc.vector.reciprocal(tmp[:], tmp[:])                 # 1/(std+eps)
    nc.vector.tensor_scalar_mul(tc_[:], tmp[:], float(alpha))
    nc.vector.tensor_mul(tmp[:], mu[:], tc_[:])
    nc.vector.tensor_scalar(uc_[:], tmp[:], -1.0, float(beta), OP.mult, OP.add)
    if apply_on_scalar:
        nc.scalar.activation(out_ap, x_ap, AF.Identity, bias=uc_[:],
                             scale=tc_[:])
    else:
        nc.vector.tensor_scalar(out_ap, x_ap, tc_[:], uc_[:], OP.mult, OP.add)


def build_program(ln1a, ln1b, ln2a, ln2b, mask_all_ones):
    nc = bacc.Bacc("TRN2", target_bir_lowering=False, debug=False)

    x_d = nc.dram_tensor("x", (S, D), F32, kind="ExternalInput").ap()
    wqT_d = nc.dram_tensor("wqT", (D, D), BF, kind="ExternalInput").ap()
    woT8_d = nc.dram_tensor("woT8", (D, D), FP8, kind="ExternalInput").ap()
    w1L_d = nc.dram_tensor("w1L", (P, FC, DC, P), BF, kind="ExternalInput").ap()
    w2L_d = nc.dram_tensor("w2L", (P, 2, FC // 2, 2, 512), BF, kind="ExternalInput").ap()
    bq_d = nc.dram_tensor("bq_v", (P, DC), F32, kind="ExternalInput").ap()
    b1_d = nc.dram_tensor("b1_v", (P, FC), F32, kind="ExternalInput").ap()
    bo_d = nc.dram_tensor("bo_rep", (P, D), F32, kind="ExternalInput").ap()
    b2_d = nc.dram_tensor("b2_rep", (P, D), F32, kind="ExternalInput").ap()
    if not mask_all_ones:
        m01_d = nc.dram_tensor("m01_v", (P, SC), F32, kind="ExternalInput").ap()
    out_d = nc.dram_tensor("out", (S, D), F32, kind="ExternalOutput").ap()

    x_r = x_d.rearrange("(sc p) d -> sc p d", p=P)
    wqT_r = wqT_d.rearrange("(kc p) o -> kc p o", p=P)
    woT8_r = woT8_d.rearrange("(oc p) d -> oc p d", p=P)
    out_r = out_d.rearrange("(sc p) d -> sc p d", p=P)

    with tile.TileContext(nc) as tc, ExitStack() as st:
        arena = st.enter_context(tc.tile_pool(name="arena", bufs=1))
        small = st.enter_context(tc.tile_pool(name="small", bufs=1))

        # ---- constants ----
        ident_b = small.tile([P, P], BF, name="ident_b")
        make_identity(nc, ident_b[:])
        ident8 = small.tile([P, P], FP8, name="ident8")
        make_identity(nc, ident8[:])
        ebias = small.tile([P, 1], F32, name="ebias")
        nc.gpsimd.memset(ebias[:], EXP_SHIFT)
        bq_sb = small.tile([P, DC], F32, name="bq_sb")
        nc.sync.dma_start(bq_sb[:], bq_d)
        b1_sb = small.tile([P, FC], F32, name="b1_sb")
        nc.sync.dma_start(b1_sb[:], b1_d)
        bo_rep = small.tile([P, D], F32, name="bo_rep")
        nc.gpsimd.dma_start(bo_rep[:], bo_d)
        b2_rep = small.tile([P, D], F32, name="b2_rep")
        nc.gpsimd.dma_start(b2_rep[:], b2_d)
        if not mask_all_ones:
            m01_sb = small.tile([P, SC], F32, name="m01_sb")
            nc.sync.dma_start(m01_sb[:], m01_d)

        # ---- persistent sbuf tiles ----
        qT8 = arena.tile([P, DC, S], FP8, tag="qT8", name="qT8")
        qh8e = arena.tile([P, SC, H * HW], FP8, tag="qh8e", name="qh8e")
        ctxN = arena.tile([P, SC, D], FP8, tag="ctxN", name="ctxN")
        res1 = arena.tile([P, SC, D], F32, tag="res1", name="res1")
        n1T = arena.tile([P, DC, S], BF, tag="n1T_ctxT8", name="n1T")
        wq_sb = arena.tile([P, DC, D], BF, tag="wq_n2t", bufs=2, name="wq_sb")
        woT8_sb = arena.tile([P, DC, D], FP8, tag="woT8", name="woT8_sb")

        # zero the Z-ones / pad cols of qh8e
        for h in range(H):
            nc.gpsimd.memset(qh8e[:, :, ds(h * HW + DK, 1)], 1.0)
            nc.gpsimd.memset(qh8e[:, :, ds(h * HW + DK + 1, 1)], 0.0)

        for kc in range(DC):
            (nc.sync if kc % 2 == 0 else nc.gpsimd).dma_start(
                wq_sb[:, kc], wqT_r[kc])
        for oc in range(DC):
            nc.gpsimd.dma_start(woT8_sb[:, oc], woT8_r[oc])

        # =========== phase 1: LN1 streamed per chunk + n1T transposes =======
        with tc.tile_pool(name="ps1", bufs=1, space="PSUM") as ps1:
            for sc in range(SC):
                xts = arena.tile([P, D], F32, tag="xts", bufs=2, name="xts")
                nc.sync.dma_start(xts[:], x_r[sc])
                n1s = arena.tile([P, D], BF, tag="n1s", bufs=2, name="n1s")
                _emit_ln_chunk(nc, small, xts[:], n1s[:], n1s[:], ln1a, ln1b)
                for cb in range(DC):
                    tpB = ps1.tile([P, P], BF, tag="tpB", bufs=4, name="tpB")
                    nc.tensor.transpose(tpB[:], n1s[:, ts(cb, P)], ident_b[:])
                    nc.vector.tensor_copy(n1T[:, cb, ts(sc, P)], tpB[:])

        # attention helpers ---------------------------------------------------
        def qproj_head(psQ, ps_tp8, hp):
            """q projection for feature chunk oc=hp -> qT8 + qh8e slices."""
            for b in range(2):
                pb = psQ.tile([P, 512], F32, tag="pb", bufs=1, name="pb")
                for kc in range(DC):
                    nc.tensor.matmul(
                        pb[:], wq_sb[:, kc, ts(hp, P)],
                        n1T[:, kc, ds(512 * b, 512)],
                        start=(kc == 0), stop=(kc == DC - 1),
                    )
                nc.vector.tensor_scalar(
                    qT8[:, hp, ds(512 * b, 512)], pb[:],
                    bq_sb[:, ds(hp, 1)], None, OP.add,
                )
            for sc in range(SC):
                tp8 = ps_tp8.tile([P, P, 2], FP8, tag="tp8", bufs=1,
                                  name="tp8")
                nc.tensor.transpose(
                    tp8[:, :, ds(0, 1)], qT8[:, hp, ts(sc, P)], ident8[:])
                for hl in range(2):
                    nc.vector.tensor_scalar_mul(
                        qh8e[:, sc, ds((2 * hp + hl) * HW, DK)],
                        tp8[:, ds(hl * DK, DK), 0], CTX_SCALE,
                    )

        def attn_head(psS, psC, hp, half):
            """scores+exp+ctx for head pair hp, query half `half`."""
            ec8 = arena.tile([P, SC, 1024], FP8, tag="ec8", bufs=2, name="ec8")
            for c in range(SC):
                sp = psS.tile([P, 1024], F32, tag="sp", bufs=2, name="sp")
                for hl in range(2):
                    lo = hl * DK
                    nc.tensor.matmul(
                        sp[:, ds(hl * 512, 512)],
                        qT8[ds(lo, DK), hp, ts(c, P)],
                        qT8[ds(lo, DK), hp, ds(512 * half, 512)],
                        start=True, stop=True,
                        tile_position=(lo, 0),
                    )
                nc.scalar.activation(
                    ec8[:, c], sp[:], AF.Exp, bias=ebias[:], scale=0.125,
                )
                if not mask_all_ones:
                    nc.vector.tensor_scalar_mul(
                        ec8[:, c], ec8[:, c], m01_sb[:, ds(c, 1)],
                    )
            for hl in range(2):
                h = 2 * hp + hl
                for qc in range(4):
                    cxp = psC.tile([P, 128], F32, tag="cxp", bufs=2,
                                   name="cxp")
                    for ci in range(SC // 2):
                        nc.tensor.matmul(
                            cxp[:, ds(0, HW)],
                            ec8[:, ds(2 * ci, 2),
                                ds(hl * 512 + qc * P, P)],
                            qh8e[:, ds(2 * ci, 2), ds(h * HW, HW)],
                            start=(ci == 0), stop=(ci == SC // 2 - 1),
                            perf_mode=DR,
                        )
                    rz = small.tile([P, 1], F32, tag="rz", bufs=3, name="rz")
                    nc.vector.reciprocal_approx_fast(rz[:], cxp[:, ds(DK, 1)])
                    nc.vector.tensor_scalar(
                        ctxN[:, half * 4 + qc, ds(h * DK, DK)],
                        cxp[:, ds(0, DK)], rz[:], None, OP.mult,
                    )

        def ctx_transpose_half(ps_tp, half, sls=range(4), pbufs=1):
            for sl in sls:
                sc = half * 4 + sl
                for oc in range(DC):
                    tpC8 = ps_tp.tile([P, P, 2], FP8, tag="tpX", bufs=pbufs,
                                      name="tpC8")
                    nc.tensor.transpose(
                        tpC8[:, :, ds(0, 1)], ctxN[:, sc, ts(oc, P)],
                        ident8[:])
                    nc.vector.tensor_copy(
                        ctxT8_t[:, oc, ts(sc, P)], tpC8[:, :, 0])

        def wo_half(psW, half, sls=range(4), pbufs=1):
            for sl in sls:
                sc = half * 4 + sl
                xre = arena.tile([P, D], F32, tag="xre", bufs=1, name="xre")
                nc.sync.dma_start(xre[:], x_r[sc])
                for dh in range(2):
                    wp = psW.tile([P, 512], F32, tag="wp", bufs=pbufs,
                                  name="wp")
                    for oi in range(DC // 2):
                        nc.tensor.matmul(
                            wp[:],
                            ctxT8_t[:, ds(2 * oi, 2), ts(sc, P)],
                            woT8_sb[:, ds(2 * oi, 2), ds(512 * dh, 512)],
                            start=(oi == 0), stop=(oi == DC // 2 - 1),
                            perf_mode=DR,
                        )
                    nc.vector.scalar_tensor_tensor(
                        res1[:, sc, ds(512 * dh, 512)], wp[:], OUT_SCALE,
                        xre[:, ds(512 * dh, 512)], OP.mult, OP.add,
                    )
                    nc.vector.tensor_add(
                        res1[:, sc, ds(512 * dh, 512)],
                        res1[:, sc, ds(512 * dh, 512)],
                        bo_rep[:, ds(512 * dh, 512)],
                    )

        def ln2_stats(half, sl):
            sc = half * 4 + sl
            n2s = arena.tile([P, D], BF, tag="n2s", bufs=2, name="n2s")
            _emit_ln_chunk(nc, small, res1[:, sc], n2s[:], n2s[:],
                           ln2a, ln2b)
            return n2s

        def ln2_transposes(ps_tp, n2s, n2Th, half, sl, pbufs=1):
            sc = half * 4 + sl
            for cb in range(DC):
                tpC = ps_tp.tile([P, P], BF, tag="tpX", bufs=pbufs,
                                 name="tpC")
                nc.tensor.transpose(tpC[:], n2s[:, ts(cb, P)], ident_b[:])
                nc.vector.tensor_copy(n2Th[:, cb, ts(sl, P)], tpC[:])
            nc.vector.tensor_add(res1[:, sc], res1[:, sc], b2_rep[:])

        def ln2_half(ps_tp, half, n2Th, sls=range(4), pbufs=1):
            for sl in sls:
                n2s = ln2_stats(half, sl)
                ln2_transposes(ps_tp, n2s, n2Th, half, sl, pbufs)

        def ffn1_chunk(psF, wsp, n2Th, h1t, lfc, fc, relu_on_scalar):
            wts = wsp.tile([P, DC, P], BF, tag="w1s", bufs=3, name="w1s")
            (nc.sync if fc % 2 == 0 else nc.gpsimd).dma_start(
                wts[:], w1L_d[:, fc])
            fp = psF.tile([P, 512], F32, tag="f1ps", bufs=2, name="f1ps")
            for dc in range(DC):
                nc.tensor.matmul(
                    fp[:], wts[:, dc], n2Th[:, dc, :],
                    start=(dc == 0), stop=(dc == DC - 1),
                )
            if relu_on_scalar:
                nc.scalar.activation(
                    h1t[:, lfc], fp[:], AF.Relu, bias=b1_sb[:, ds(fc, 1)],
                )
            else:
                nc.vector.tensor_scalar(
                    h1t[:, lfc], fp[:], b1_sb[:, ds(fc, 1)], 0.0,
                    OP.add, OP.max,
                )

        def ffn2_drain(half, dh, ops):
            for sl in range(4):
                sc = half * 4 + sl
                nc.vector.tensor_add(
                    res1[:, sc, ds(512 * dh, 512)], ops[sl][:],
                    res1[:, sc, ds(512 * dh, 512)],
                )
                (nc.gpsimd if sl % 2 == 0 else nc.scalar).dma_start(
                    out_r[sc][:, ds(512 * dh, 512)],
                    res1[:, sc, ds(512 * dh, 512)],
                )

        def ffn2_mms(ops, h1at, w2t, fc2):
            for fi in range(2):
                fc = 2 * fc2 + fi
                h1t, lfc = h1at(fc)
                for sl in range(4):
                    nc.tensor.matmul(
                        ops[sl][:], h1t[:, lfc, ts(sl, P)], w2t[:, fi],
                        start=(fc == 0), stop=(fc == FC - 1),
                    )

        # ================== phase 2: qproj pipeline + attention half 0 ======
        ctxT8_t = None
        with tc.tile_pool(name="wstream", bufs=1) as wsp:
            with tc.tile_pool(name="psSp", bufs=1, space="PSUM") as psS, \
                 tc.tile_pool(name="psCx", bufs=1, space="PSUM") as psC:
                with tc.tile_pool(name="psQ", bufs=1, space="PSUM") as psQ:
                    for hp in range(DC):
                        qproj_head(psQ, psQ, hp)
                        if hp >= 1:
                            attn_head(psS, psC, hp - 1, 0)
                    attn_head(psS, psC, DC - 1, 0)

                # ========= phase 3: ctxT8(0), wo(0), ln2(0) =================
                ctxT8_t = arena.tile([P, DC, S], FP8, tag="n1T_ctxT8",
                                     name="ctxT8")
                n2Th0 = arena.tile([P, DC, 512], BF, tag="wq_n2t",
                                   bufs=2, name="n2Th0")
                with tc.tile_pool(name="ps3", bufs=1, space="PSUM") as ps3:
                    ctx_transpose_half(ps3, 0)
                    wo_half(ps3, 0)
                    ln2_half(ps3, 0, n2Th0)

                # ==== phase 4: attention half 1 with ffn1(0) interleaved ====
                h1a = arena.tile([P, FC, 512], BF, tag="xt_h1", name="h1a")
                with tc.tile_pool(name="ps4", bufs=1, space="PSUM") as ps4:
                    for hp in range(DC):
                        attn_head(psS, psC, hp, 1)
                        for fc in range(4 * hp, 4 * hp + 4):
                            ffn1_chunk(ps4, wsp, n2Th0, h1a, fc, fc, False)

            # ==== phase 5 + pass A: ctxT8(1)/wo(1)/ln2(1) and ffn1(1),
            # with ffn2(half0, dh0) matmuls injected as PE filler ============
            n2Th1 = arena.tile([P, DC, 512], BF, tag="wq_n2t", bufs=2,
                               name="n2Th1")
            h1b_parts = [
                arena.tile([P, 8, 512], BF, tag="ec8", bufs=2, name="h1b0"),
                arena.tile([P, 8, 512], BF, tag="ec8", bufs=2, name="h1b1"),
                arena.tile([P, 8, 512], BF, tag="qT8", name="h1b2"),
                arena.tile([P, 8, 512], BF, tag="qh8e", name="h1b3"),
            ]
            h1a_at = lambda fc: (h1a, fc)
            h1b_at = lambda fc: (h1b_parts[fc // 8], fc % 8)

            with tc.tile_pool(name="psT6", bufs=1, space="PSUM") as psT6:
                opsA = [psT6.tile([P, 512], F32, tag="f2psA", bufs=4,
                                  name="f2psA") for _ in range(4)]

                def ffn2A_chunk(fc2):
                    w2t = wsp.tile([P, 2, 512], BF, tag="w2s", bufs=3,
                                   name="w2s")
                    (nc.sync if fc2 % 2 == 0 else nc.scalar).dma_start(
                        w2t[:], w2L_d[:, 0, fc2])
                    ffn2_mms(opsA, h1a_at, w2t, fc2)

                with tc.tile_pool(name="ps5", bufs=1, space="PSUM") as ps5:
                    for sl in range(4):
                        ctx_transpose_half(ps5, 1, [sl], pbufs=2)
                        wo_half(ps5, 1, [sl], pbufs=2)
                        n2s = ln2_stats(1, sl)
                        ffn2A_chunk(2 * sl)
                        ffn2A_chunk(2 * sl + 1)
                        ln2_transposes(ps5, n2s, n2Th1, 1, sl, pbufs=2)
                with tc.tile_pool(name="psA6", bufs=1, space="PSUM") as psA6:
                    for fc2 in range(8, FC // 2):
                        ffn2A_chunk(fc2)
                        for fc in range(4 * (fc2 - 8), 4 * (fc2 - 8) + 4):
                            h1t, lfc = h1b_at(fc)
                            ffn1_chunk(psA6, wsp, n2Th1, h1t, lfc, fc, True)
                ffn2_drain(0, 0, opsA)

            # pass B: ffn2(half0, dh1) + ffn2(half1, dh1), shared w2 stream
            with tc.tile_pool(name="psB6", bufs=1, space="PSUM") as psB6:
                opsB0 = [psB6.tile([P, 512], F32, tag="f2psB0", bufs=4,
                                   name="f2psB0") for _ in range(4)]
                opsB1 = [psB6.tile([P, 512], F32, tag="f2psB1", bufs=4,
                                   name="f2psB1") for _ in range(4)]
                for fc2 in range(FC // 2):
                    w2t = wsp.tile([P, 2, 512], BF, tag="w2s", bufs=3,
                                   name="w2s")
                    (nc.sync if fc2 % 2 == 0 else nc.scalar).dma_start(
                        w2t[:], w2L_d[:, 1, fc2])
                    ffn2_mms(opsB0, h1a_at, w2t, fc2)
                    ffn2_mms(opsB1, h1b_at, w2t, fc2)
                ffn2_drain(0, 1, opsB0)
                ffn2_drain(1, 1, opsB1)

            # pass C: ffn2(half1, dh0)
            with tc.tile_pool(name="psC6", bufs=1, space="PSUM") as psC6:
                opsC = [psC6.tile([P, 512], F32, tag="f2psC", bufs=4,
                                  name="f2psC") for _ in range(4)]
                for fc2 in range(FC // 2):
                    w2t = wsp.tile([P, 2, 512], BF, tag="w2s", bufs=3,
                                   name="w2s")
                    (nc.sync if fc2 % 2 == 0 else nc.scalar).dma_start(
                        w2t[:], w2L_d[:, 0, fc2])
                    ffn2_mms(opsC, h1b_at, w2t, fc2)
                ffn2_drain(1, 0, opsC)

    nc.compile()
    return nc


def _prep_inputs(inputs):
    f32 = lambda a: np.ascontiguousarray(np.asarray(a, dtype=np.float32))
    bfT = lambda a: np.ascontiguousarray(
        np.asarray(a, dtype=np.float32).T.astype(ml_dtypes.bfloat16))
    x = f32(inputs["x"])                      # [B, S, D]
    mask = np.asarray(inputs["src_mask"])     # [B, 1, 1, S] int32
    wqT = bfT(inputs["wq"])                   # [D, D] (in, out)
    woT8 = np.ascontiguousarray(
        (np.asarray(inputs["wo"], dtype=np.float32).T * WO_SCALE)
        .astype(ml_dtypes.float8_e4m3))
    w1 = np.asarray(inputs["w1"], dtype=np.float32)      # [DFF, D]
    w2 = np.asarray(inputs["w2"], dtype=np.float32)      # [D, DFF]
    # w1L[p, fc, dc, f] = w1[fc*128+f, dc*128+p]; 2KB-contiguous DMA chunks
    w1L = np.ascontiguousarray(
        w1.reshape(FC, P, DC, P).transpose(3, 0, 2, 1)
        .astype(ml_dtypes.bfloat16))
    # w2L[p, dh, fc2, i, d] = w2[dh*512+d, (2*fc2+i)*128+p]
    w2L = np.ascontiguousarray(
        w2.reshape(2, 512, FC // 2, 2, P).transpose(4, 0, 2, 3, 1)
        .astype(ml_dtypes.bfloat16))
    bq_v = np.ascontiguousarray(f32(inputs["bq"]).reshape(DC, P).T)
    b1_v = np.ascontiguousarray(f32(inputs["b1"]).reshape(FC, P).T)
    bo_rep = np.ascontiguousarray(np.tile(f32(inputs["bo"]), (P, 1)))
    b2_rep = np.ascontiguousarray(np.tile(f32(inputs["b2"]), (P, 1)))
    scal = lambda k: float(np.asarray(inputs[k]).reshape(-1)[0])
    ln = (scal("ln1_a"), scal("ln1_b"), scal("ln2_a"), scal("ln2_b"))
    mask_all_ones = bool((mask != 0).all())

    shared = dict(wqT=wqT, woT8=woT8, w1L=w1L, w2L=w2L, bq_v=bq_v, b1_v=b1_v,
                  bo_rep=bo_rep, b2_rep=b2_rep)
    in_maps = []
    for b in range(NB):
        m = dict(shared)
        m["x"] = np.ascontiguousarray(x[b])
        if not mask_all_ones:
            m01 = (mask[b].reshape(S) != 0).astype(np.float32)
            m["m01_v"] = np.ascontiguousarray(m01.reshape(SC, P).T)
        in_maps.append(m)
    return in_maps, ln, mask_all_ones


last_nc = None
last_in_maps = None


def kernel(**inputs):
    global last_nc, last_in_maps
    in_maps, ln, mask_all_ones = _prep_inputs(inputs)
    nc = build_program(*ln, mask_all_ones)
    last_nc, last_in_maps = nc, in_maps
    res = bass_utils.run_bass_kernel_spmd(
        nc, in_maps, core_ids=list(range(NB)), trace=False,
    )
    out = np.stack([np.asarray(res.results[b]["out"]) for b in range(NB)])
    return out.astype(np.float32)


# revision 19
# speedup vs baseline: 1.0769x; 1.0015x over previous
"""Trainium2 Bass kernel for nn_EncoderBlock (dense transformer encoder block).

Data parallel: batch B=8 across 8 NeuronCores, one element per core.

v2 design vs v1 baseline (685us):
  - scores matmuls in fp8 DoubleRow (zero-padded K=64 subtile): 2x.
  - ctx computed "flipped" (out = [queries, feats]) in fp8 DoubleRow with a
    fused ones-column producing the softmax normalizer Z in the same psum
    tile; per-partition normalize via reciprocal_approx_fast + tensor_scalar.
    Kills the ones-matmul for Z and the replicated [128,512] reciprocal.
  - wo in fp8 DoubleRow; wo weights pre-scaled x64 host-side and ctx scaled
    x16 on-chip (fp8 subnormal avoidance), un-scaled by 1/1024 in the
    residual add.
  - qproj per head-pair pipelined under the attention-half-0 exp stream;
    ffn1(half 0) interleaved under the attention-half-1 exp stream.
  - LN sum/sum-of-squares both via ScalarE ACT accumulate (Copy / Square).
FFN and qproj matmuls stay bf16 (fp8 there would blow the 2e-2 error gate).
"""

import sys

sys.path.insert(0, "/opt/trn_rl_repo")

import numpy as np
import ml_dtypes
from contextlib import ExitStack

import concourse.bass as bass
import concourse.tile as tile
from concourse import bacc, mybir
from concourse import bass_utils
from concourse.bass import ts, ds
from concourse.masks import make_identity

BF = mybir.dt.bfloat16
F32 = mybir.dt.float32
FP8 = mybir.dt.float8e4
AF = mybir.ActivationFunctionType
OP = mybir.AluOpType
DR = mybir.MatmulPerfMode.DoubleRow

P = 128
S = 1024          # sequence length per core
D = 1024          # d_model
H = 16            # heads
DK = 64           # head dim
DFF = 4096
NB = 8            # batch = number of cores
SC = S // P       # 8 sequence chunks
DC = D // P       # 8 feature chunks
FC = DFF // P     # 32 ff chunks
EPS = 1e-6
EXP_SHIFT = -2.0   # constant shift inside exp; cancels in softmax ratio
CTX_SCALE = 16.0   # on-chip scale of v (=q) into qh8e; keeps ctx out of
WO_SCALE = 64.0    # fp8 subnormal range.  woT8 = fp8(64*wo) host-side.
OUT_SCALE = 1.0 / (CTX_SCALE * WO_SCALE)
HW = 66            # per-head stride in qh8e: 64 feats + Z-ones col + pad

last_exec_time_ns = None


def _emit_ln_chunk(nc, small, x_ap, out_ap, scratch_ap, alpha, beta,
                   apply_engine=None):
    """Bessel-corrected LN of one [P, D] chunk, stats per token on partitions.
    n = (x - mu)/(std + eps)*alpha + beta.  Sum and sum-of-squares come from
    ScalarE ACT accumulate (Copy / Square) writing scratch_ap; scratch is
    overwritten by the final apply (out_ap may alias scratch_ap)."""
    s1 = small.tile([P, 1], F32, tag="ln_s1", bufs=3, name="ln_s1")
    sq = small.tile([P, 1], F32, tag="ln_sq", bufs=3, name="ln_sq")
    mu = small.tile([P, 1], F32, tag="ln_mu", bufs=3, name="ln_mu")
    var = small.tile([P, 1], F32, tag="ln_var", bufs=3, name="ln_var")
    tmp = small.tile([P, 1], F32, tag="ln_tmp", bufs=3, name="ln_tmp")
    s0 = small.tile([P, 1], F32, tag="ln_s0", bufs=3, name="ln_s0")
    tc_ = small.tile([P, 1], F32, tag="ln_tc", bufs=3, name="ln_tc")
    uc_ = small.tile([P, 1], F32, tag="ln_uc", bufs=3, name="ln_uc")

    nc.scalar.activation(scratch_ap, x_ap, AF.Copy, accum_out=s1[:])
    nc.scalar.activation(scratch_ap, x_ap, AF.Square, accum_out=sq[:])
    nc.vector.tensor_scalar_mul(mu[:], s1[:], 1.0 / D)
    nc.vector.tensor_mul(tmp[:], mu[:], mu[:])
    nc.vector.tensor_scalar_mul(var[:], sq[:], 1.0 / (D - 1))
    nc.vector.tensor_scalar_mul(tmp[:], tmp[:], float(D) / (D - 1))
    nc.vector.tensor_sub(var[:], var[:], tmp[:])
    # std = sqrt(var): ACT sqrt + one Newton step  s1 = 0.5*(s0 + var/s0)
    nc.scalar.activation(s0[:], var[:], AF.Sqrt)
    nc.vector.reciprocal(tmp[:], s0[:])
    nc.vector.tensor_mul(tmp[:], tmp[:], var[:])
    nc.vector.tensor_add(tmp[:], tmp[:], s0[:])
    nc.vector.tensor_scalar(tmp[:], tmp[:], 0.5, EPS, OP.mult, OP.add)
    nc.vector.reciprocal(tmp[:], tmp[:])                 # 1/(std+eps)
    nc.vector.tensor_scalar_mul(tc_[:], tmp[:], float(alpha))
    nc.vector.tensor_mul(tmp[:], mu[:], tc_[:])
    nc.vector.tensor_scalar(uc_[:], tmp[:], -1.0, float(beta), OP.mult, OP.add)
    eng = apply_engine if apply_engine is not None else nc.vector
    eng.tensor_scalar(out_ap, x_ap, tc_[:], uc_[:], OP.mult, OP.add)


def build_program(ln1a, ln1b, ln2a, ln2b, mask_all_ones):
    nc = bacc.Bacc("TRN2", target_bir_lowering=False, debug=False)

    x_d = nc.dram_tensor("x", (S, D), F32, kind="ExternalInput").ap()
    wqT_d = nc.dram_tensor("wqT", (D, D), BF, kind="ExternalInput").ap()
    woT8_d = nc.dram_tensor("woT8", (D, D), FP8, kind="ExternalInput").ap()
    w1L_d = nc.dram_tensor("w1L", (P, FC, DC, P), BF, kind="ExternalInput").ap()
    w2L_d = nc.dram_tensor("w2L", (P, 2, FC // 2, 2, 512), BF, kind="ExternalInput").ap()
    bq_d = nc.dram_tensor("bq_v", (P, DC), F32, kind="ExternalInput").ap()
    b1_d = nc.dram_tensor("b1_v", (P, FC), F32, kind="ExternalInput").ap()
    bo_d = nc.dram_tensor("bo_rep", (P, D), F32, kind="ExternalInput").ap()
    b2_d = nc.dram_tensor("b2_rep", (P, D), F32, kind="ExternalInput").ap()
    if not mask_all_ones:
        m01_d = nc.dram_tensor("m01_v", (P, SC), F32, kind="ExternalInput").ap()
    out_d = nc.dram_tensor("out", (S, D), F32, kind="ExternalOutput").ap()

    x_r = x_d.rearrange("(sc p) d -> sc p d", p=P)
    wqT_r = wqT_d.rearrange("(kc p) o -> kc p o", p=P)
    woT8_r = woT8_d.rearrange("(oc p) d -> oc p d", p=P)
    out_r = out_d.rearrange("(sc p) d -> sc p d", p=P)

    with tile.TileContext(nc) as tc, ExitStack() as st:
        arena = st.enter_context(tc.tile_pool(name="arena", bufs=1))
        small = st.enter_context(tc.tile_pool(name="small", bufs=1))

        # ---- constants ----
        ident_b = small.tile([P, P], BF, name="ident_b")
        make_identity(nc, ident_b[:])
        ident8 = small.tile([P, P], FP8, name="ident8")
        make_identity(nc, ident8[:])
        ebias = small.tile([P, 1], F32, name="ebias")
        nc.gpsimd.memset(ebias[:], EXP_SHIFT)
        bq_sb = small.tile([P, DC], F32, name="bq_sb")
        nc.sync.dma_start(bq_sb[:], bq_d)
        b1_sb = small.tile([P, FC], F32, name="b1_sb")
        nc.sync.dma_start(b1_sb[:], b1_d)
        bo_rep = small.tile([P, D], F32, name="bo_rep")
        nc.gpsimd.dma_start(bo_rep[:], bo_d)
        b2_rep = small.tile([P, D], F32, name="b2_rep")
        nc.gpsimd.dma_start(b2_rep[:], b2_d)
        if not mask_all_ones:
            m01_sb = small.tile([P, SC], F32, name="m01_sb")
            nc.sync.dma_start(m01_sb[:], m01_d)

        # ---- persistent sbuf tiles ----
        qT8 = arena.tile([P, DC, S], FP8, tag="qT8", name="qT8")
        qh8e = arena.tile([P, SC, H * HW], FP8, tag="qh8e", name="qh8e")
        ctxN = arena.tile([P, SC, D], FP8, tag="ctxN", name="ctxN")
        res1 = arena.tile([P, SC, D], F32, tag="res1", name="res1")
        n1T = arena.tile([P, DC, S], BF, tag="n1T_ctxT8", name="n1T")
        wq_sb = arena.tile([P, DC, D], BF, tag="wq_n2t", bufs=2, name="wq_sb")
        woT8_sb = arena.tile([P, DC, D], FP8, tag="woT8", name="woT8_sb")

        # zero the Z-ones / pad cols of qh8e
        for h in range(H):
            nc.gpsimd.memset(qh8e[:, :, ds(h * HW + DK, 1)], 1.0)
            nc.gpsimd.memset(qh8e[:, :, ds(h * HW + DK + 1, 1)], 0.0)

        for kc in range(DC):
            (nc.sync if kc % 2 == 0 else nc.gpsimd).dma_start(
                wq_sb[:, kc], wqT_r[kc])
        for oc in range(DC):
            nc.gpsimd.dma_start(woT8_sb[:, oc], woT8_r[oc])

        # =========== phase 1: LN1 streamed per chunk + n1T transposes =======
        with tc.tile_pool(name="ps1", bufs=1, space="PSUM") as ps1:
            for sc in range(SC):
                xts = arena.tile([P, D], F32, tag="xts", bufs=2, name="xts")
                nc.sync.dma_start(xts[:], x_r[sc])
                n1s = arena.tile([P, D], BF, tag="n1s", bufs=3, name="n1s")
                _emit_ln_chunk(nc, small, xts[:], n1s[:], n1s[:], ln1a, ln1b,
                               apply_engine=nc.gpsimd)
                for cb in range(DC):
                    tpB = ps1.tile([P, P], BF, tag="tpB", bufs=4, name="tpB")
                    nc.tensor.transpose(tpB[:], n1s[:, ts(cb, P)], ident_b[:])
                    nc.vector.tensor_copy(n1T[:, cb, ts(sc, P)], tpB[:])

        # attention helpers ---------------------------------------------------
        def qproj_head(psQ, ps_tp8, hp):
            """q projection for feature chunk oc=hp -> qT8 + qh8e slices."""
            for b in range(2):
                pb = psQ.tile([P, 512], F32, tag="pb", bufs=1, name="pb")
                for kc in range(DC):
                    nc.tensor.matmul(
                        pb[:], wq_sb[:, kc, ts(hp, P)],
                        n1T[:, kc, ds(512 * b, 512)],
                        start=(kc == 0), stop=(kc == DC - 1),
                    )
                nc.vector.tensor_scalar(
                    qT8[:, hp, ds(512 * b, 512)], pb[:],
                    bq_sb[:, ds(hp, 1)], None, OP.add,
                )
            for sc in range(SC):
                tp8 = ps_tp8.tile([P, P, 2], FP8, tag="tp8", bufs=1,
                                  name="tp8")
                nc.tensor.transpose(
                    tp8[:, :, ds(0, 1)], qT8[:, hp, ts(sc, P)], ident8[:])
                for hl in range(2):
                    nc.vector.tensor_scalar_mul(
                        qh8e[:, sc, ds((2 * hp + hl) * HW, DK)],
                        tp8[:, ds(hl * DK, DK), 0], CTX_SCALE,
                    )

        def attn_head(psS, psC, hp, half):
            """scores+exp+ctx for head pair hp, query half `half`."""
            ec8 = arena.tile([P, SC, 1024], FP8, tag="ec8", bufs=2, name="ec8")
            for c in range(SC):
                sp = psS.tile([P, 1024], F32, tag="sp", bufs=2, name="sp")
                for hl in range(2):
                    lo = hl * DK
                    nc.tensor.matmul(
                        sp[:, ds(hl * 512, 512)],
                        qT8[ds(lo, DK), hp, ts(c, P)],
                        qT8[ds(lo, DK), hp, ds(512 * half, 512)],
                        start=True, stop=True,
                        tile_position=(lo, 0),
                    )
                nc.scalar.activation(
                    ec8[:, c], sp[:], AF.Exp, bias=ebias[:], scale=0.125,
                )
                if not mask_all_ones:
                    nc.vector.tensor_scalar_mul(
                        ec8[:, c], ec8[:, c], m01_sb[:, ds(c, 1)],
                    )
            for hl in range(2):
                h = 2 * hp + hl
                for qc in range(4):
                    cxp = psC.tile([P, 128], F32, tag="cxp", bufs=2,
                                   name="cxp")
                    for ci in range(SC // 2):
                        nc.tensor.matmul(
                            cxp[:, ds(0, HW)],
                            ec8[:, ds(2 * ci, 2),
                                ds(hl * 512 + qc * P, P)],
                            qh8e[:, ds(2 * ci, 2), ds(h * HW, HW)],
                            start=(ci == 0), stop=(ci == SC // 2 - 1),
                            perf_mode=DR,
                        )
                    rz = small.tile([P, 1], F32, tag="rz", bufs=3, name="rz")
                    nc.vector.reciprocal_approx_fast(rz[:], cxp[:, ds(DK, 1)])
                    nc.vector.tensor_scalar(
                        ctxN[:, half * 4 + qc, ds(h * DK, DK)],
                        cxp[:, ds(0, DK)], rz[:], None, OP.mult,
                    )

        def ctx_transpose_half(ps_tp, half, sls=range(4), pbufs=1):
            for sl in sls:
                sc = half * 4 + sl
                for oc in range(DC):
                    tpC8 = ps_tp.tile([P, P, 2], FP8, tag="tpX", bufs=pbufs,
                                      name="tpC8")
                    nc.tensor.transpose(
                        tpC8[:, :, ds(0, 1)], ctxN[:, sc, ts(oc, P)],
                        ident8[:])
                    nc.vector.tensor_copy(
                        ctxT8_t[:, oc, ts(sc, P)], tpC8[:, :, 0])

        def wo_half(psW, half, sls=range(4), pbufs=1):
            for sl in sls:
                sc = half * 4 + sl
                xre = arena.tile([P, D], F32, tag="xre", bufs=1, name="xre")
                nc.sync.dma_start(xre[:], x_r[sc])
                for dh in range(2):
                    wp = psW.tile([P, 512], F32, tag="wp", bufs=pbufs,
                                  name="wp")
                    for oi in range(DC // 2):
                        nc.tensor.matmul(
                            wp[:],
                            ctxT8_t[:, ds(2 * oi, 2), ts(sc, P)],
                            woT8_sb[:, ds(2 * oi, 2), ds(512 * dh, 512)],
                            start=(oi == 0), stop=(oi == DC // 2 - 1),
                            perf_mode=DR,
                        )
                    nc.vector.scalar_tensor_tensor(
                        res1[:, sc, ds(512 * dh, 512)], wp[:], OUT_SCALE,
                        xre[:, ds(512 * dh, 512)], OP.mult, OP.add,
                    )
                    nc.vector.tensor_add(
                        res1[:, sc, ds(512 * dh, 512)],
                        res1[:, sc, ds(512 * dh, 512)],
                        bo_rep[:, ds(512 * dh, 512)],
                    )

        def ln2_stats(half, sl):
            sc = half * 4 + sl
            n2s = arena.tile([P, D], BF, tag="n2s", bufs=4, name="n2s")
            _emit_ln_chunk(nc, small, res1[:, sc], n2s[:], n2s[:],
                           ln2a, ln2b)
            return n2s

        def ln2_transposes(ps_tp, n2s, n2Th, half, sl, pbufs=1):
            sc = half * 4 + sl
            for cb in range(DC):
                tpC = ps_tp.tile([P, P], BF, tag="tpX", bufs=pbufs,
                                 name="tpC")
                nc.tensor.transpose(tpC[:], n2s[:, ts(cb, P)], ident_b[:])
                nc.vector.tensor_copy(n2Th[:, cb, ts(sl, P)], tpC[:])
            nc.vector.tensor_add(res1[:, sc], res1[:, sc], b2_rep[:])

        def ln2_half(ps_tp, half, n2Th, sls=range(4), pbufs=1):
            for sl in sls:
                n2s = ln2_stats(half, sl)
                ln2_transposes(ps_tp, n2s, n2Th, half, sl, pbufs)

        def ffn1_chunk(psF, wsp, n2Th, h1t, lfc, fc, relu_on_scalar):
            wts = wsp.tile([P, DC, P], BF, tag="w1s", bufs=3, name="w1s")
            (nc.sync if fc % 2 == 0 else nc.gpsimd).dma_start(
                wts[:], w1L_d[:, fc])
            fp = psF.tile([P, 512], F32, tag="f1ps", bufs=2, name="f1ps")
            for dc in range(DC):
                nc.tensor.matmul(
                    fp[:], wts[:, dc], n2Th[:, dc, :],
                    start=(dc == 0), stop=(dc == DC - 1),
                )
            if relu_on_scalar:
                nc.scalar.activation(
                    h1t[:, lfc], fp[:], AF.Relu, bias=b1_sb[:, ds(fc, 1)],
                )
            else:
                nc.vector.tensor_scalar(
                    h1t[:, lfc], fp[:], b1_sb[:, ds(fc, 1)], 0.0,
                    OP.add, OP.max,
                )

        def ffn2_drain(half, dh, ops):
            for sl in range(4):
                sc = half * 4 + sl
                nc.vector.tensor_add(
                    res1[:, sc, ds(512 * dh, 512)], ops[sl][:],
                    res1[:, sc, ds(512 * dh, 512)],
                )
                (nc.gpsimd if sl % 2 == 0 else nc.scalar).dma_start(
                    out_r[sc][:, ds(512 * dh, 512)],
                    res1[:, sc, ds(512 * dh, 512)],
                )

        def ffn2_mms(ops, h1at, w2t, fc2):
            for fi in range(2):
                fc = 2 * fc2 + fi
                h1t, lfc = h1at(fc)
                for sl in range(4):
                    nc.tensor.matmul(
                        ops[sl][:], h1t[:, lfc, ts(sl, P)], w2t[:, fi],
                        start=(fc == 0), stop=(fc == FC - 1),
                    )

        # ================== phase 2: qproj pipeline + attention half 0 ======
        ctxT8_t = None
        with tc.tile_pool(name="wstream", bufs=1) as wsp:
            with tc.tile_pool(name="psSp", bufs=1, space="PSUM") as psS, \
                 tc.tile_pool(name="psCx", bufs=1, space="PSUM") as psC:
                with tc.tile_pool(name="psQ", bufs=1, space="PSUM") as psQ:
                    for hp in range(DC):
                        qproj_head(psQ, psQ, hp)
                        if hp >= 1:
                            attn_head(psS, psC, hp - 1, 0)
                    attn_head(psS, psC, DC - 1, 0)

                # ========= phase 3: ctxT8(0), wo(0), ln2(0) =================
                ctxT8_t = arena.tile([P, DC, S], FP8, tag="n1T_ctxT8",
                                     name="ctxT8")
                n2Th0 = arena.tile([P, DC, 512], BF, tag="wq_n2t",
                                   bufs=2, name="n2Th0")
                with tc.tile_pool(name="ps3", bufs=1, space="PSUM") as ps3:
                    ctx_transpose_half(ps3, 0)
                    wo_half(ps3, 0)
                    n2ss = [ln2_stats(0, sl) for sl in range(4)]
                    for sl in range(4):
                        attn_head(psS, psC, sl, 1)
                        ln2_transposes(ps3, n2ss[sl], n2Th0, 0, sl)

                # ==== phase 4: attention half 1 tail + ffn1(0) interleaved ==
                h1a = arena.tile([P, FC, 512], BF, tag="xt_h1", name="h1a")
                with tc.tile_pool(name="ps4", bufs=1, space="PSUM") as ps4:
                    for hp in range(4, DC):
                        attn_head(psS, psC, hp, 1)
                        for fc in range(8 * (hp - 4), 8 * (hp - 4) + 8):
                            ffn1_chunk(ps4, wsp, n2Th0, h1a, fc, fc, False)

            # ==== phase 5 + pass A: ctxT8(1)/wo(1)/ln2(1) and ffn1(1),
            # with ffn2(half0, dh0) matmuls injected as PE filler ============
            n2Th1 = arena.tile([P, DC, 512], BF, tag="wq_n2t", bufs=2,
                               name="n2Th1")
            h1b_parts = [
                arena.tile([P, 8, 512], BF, tag="ec8", bufs=2, name="h1b0"),
                arena.tile([P, 8, 512], BF, tag="ec8", bufs=2, name="h1b1"),
                arena.tile([P, 8, 512], BF, tag="qT8", name="h1b2"),
                arena.tile([P, 8, 512], BF, tag="qh8e", name="h1b3"),
            ]
            h1a_at = lambda fc: (h1a, fc)
            h1b_at = lambda fc: (h1b_parts[fc // 8], fc % 8)

            with tc.tile_pool(name="psT6", bufs=1, space="PSUM") as psT6:
                opsA = [psT6.tile([P, 512], F32, tag="f2psA", bufs=4,
                                  name="f2psA") for _ in range(4)]

                def ffn2A_chunk(fc2):
                    w2t = wsp.tile([P, 2, 512], BF, tag="w2s", bufs=3,
                                   name="w2s")
                    (nc.sync if fc2 % 2 == 0 else nc.scalar).dma_start(
                        w2t[:], w2L_d[:, 0, fc2])
                    ffn2_mms(opsA, h1a_at, w2t, fc2)

                with tc.tile_pool(name="ps5", bufs=1, space="PSUM") as ps5:
                    for sl in range(4):
                        ctx_transpose_half(ps5, 1, [sl], pbufs=2)
                        wo_half(ps5, 1, [sl], pbufs=2)
                        n2s = ln2_stats(1, sl)
                        ffn2A_chunk(2 * sl)
                        ffn2A_chunk(2 * sl + 1)
                        ln2_transposes(ps5, n2s, n2Th1, 1, sl, pbufs=2)
                with tc.tile_pool(name="psA6", bufs=1, space="PSUM") as psA6:
                    for fc2 in range(8, FC // 2):
                        ffn2A_chunk(fc2)
                    for fc in range(FC):
                        h1t, lfc = h1b_at(fc)
                        ffn1_chunk(psA6, wsp, n2Th1, h1t, lfc, fc, True)
                ffn2_drain(0, 0, opsA)

            # pass B: ffn2(half0, dh1) + ffn2(half1, dh1), shared w2 stream
            with tc.tile_pool(name="psB6", bufs=1, space="PSUM") as psB6:
                opsB0 = [psB6.tile([P, 512], F32, tag="f2psB0", bufs=4,
                                   name="f2psB0") for _ in range(4)]
                opsB1 = [psB6.tile([P, 512], F32, tag="f2psB1", bufs=4,
                                   name="f2psB1") for _ in range(4)]
                for fc2 in range(FC // 2):
                    w2t = wsp.tile([P, 2, 512], BF, tag="w2s", bufs=3,
                                   name="w2s")
                    (nc.sync if fc2 % 2 == 0 else nc.scalar).dma_start(
                        w2t[:], w2L_d[:, 1, fc2])
                    ffn2_mms(opsB0, h1a_at, w2t, fc2)
                    ffn2_mms(opsB1, h1b_at, w2t, fc2)
                ffn2_drain(0, 1, opsB0)
                ffn2_drain(1, 1, opsB1)

            # pass C: ffn2(half1, dh0)
            with tc.tile_pool(name="psC6", bufs=1, space="PSUM") as psC6:
                opsC = [psC6.tile([P, 512], F32, tag="f2psC", bufs=4,
                                  name="f2psC") for _ in range(4)]
                for fc2 in range(FC // 2):
                    w2t = wsp.tile([P, 2, 512], BF, tag="w2s", bufs=3,
                                   name="w2s")
                    (nc.sync if fc2 % 2 == 0 else nc.scalar).dma_start(
                        w2t[:], w2L_d[:, 0, fc2])
                    ffn2_mms(opsC, h1b_at, w2t, fc2)
                ffn2_drain(1, 0, opsC)

    nc.compile()
    return nc


def _prep_inputs(inputs):
    f32 = lambda a: np.ascontiguousarray(np.asarray(a, dtype=np.float32))
    bfT = lambda a: np.ascontiguousarray(
        np.asarray(a, dtype=np.float32).T.astype(ml_dtypes.bfloat16))
    x = f32(inputs["x"])                      # [B, S, D]
    mask = np.asarray(inputs["src_mask"])     # [B, 1, 1, S] int32
    wqT = bfT(inputs["wq"])                   # [D, D] (in, out)
    woT8 = np.ascontiguousarray(
        (np.asarray(inputs["wo"], dtype=np.float32).T * WO_SCALE)
        .astype(ml_dtypes.float8_e4m3))
    w1 = np.asarray(inputs["w1"], dtype=np.float32)      # [DFF, D]
    w2 = np.asarray(inputs["w2"], dtype=np.float32)      # [D, DFF]
    # w1L[p, fc, dc, f] = w1[fc*128+f, dc*128+p]; 2KB-contiguous DMA chunks
    w1L = np.ascontiguousarray(
        w1.reshape(FC, P, DC, P).transpose(3, 0, 2, 1)
        .astype(ml_dtypes.bfloat16))
    # w2L[p, dh, fc2, i, d] = w2[dh*512+d, (2*fc2+i)*128+p]
    w2L = np.ascontiguousarray(
        w2.reshape(2, 512, FC // 2, 2, P).transpose(4, 0, 2, 3, 1)
        .astype(ml_dtypes.bfloat16))
    bq_v = np.ascontiguousarray(f32(inputs["bq"]).reshape(DC, P).T)
    b1_v = np.ascontiguousarray(f32(inputs["b1"]).reshape(FC, P).T)
    bo_rep = np.ascontiguousarray(np.tile(f32(inputs["bo"]), (P, 1)))
    b2_rep = np.ascontiguousarray(np.tile(f32(inputs["b2"]), (P, 1)))
    scal = lambda k: float(np.asarray(inputs[k]).reshape(-1)[0])
    ln = (scal("ln1_a"), scal("ln1_b"), scal("ln2_a"), scal("ln2_b"))
    mask_all_ones = bool((mask != 0).all())

    shared = dict(wqT=wqT, woT8=woT8, w1L=w1L, w2L=w2L, bq_v=bq_v, b1_v=b1_v,
                  bo_rep=bo_rep, b2_rep=b2_rep)
    in_maps = []
    for b in range(NB):
        m = dict(shared)
        m["x"] = np.ascontiguousarray(x[b])
        if not mask_all_ones:
            m01 = (mask[b].reshape(S) != 0).astype(np.float32)
            m["m01_v"] = np.ascontiguousarray(m01.reshape(SC, P).T)
        in_maps.append(m)
    return in_maps, ln, mask_all_ones


last_nc = None
last_in_maps = None


def kernel(**inputs):
    global last_nc, last_in_maps
    in_maps, ln, mask_all_ones = _prep_inputs(inputs)
    nc = build_program(*ln, mask_all_ones)
    last_nc, last_in_maps = nc, in_maps
    res = bass_utils.run_bass_kernel_spmd(
        nc, in_maps, core_ids=list(range(NB)), trace=False,
    )
    out = np.stack([np.asarray(res.results[b]["out"]) for b in range(NB)])
    return out.astype(np.float32)
